# revision 1
# baseline (speedup 1.0000x reference)
"""GCNII node regressor on 8 trn2 NeuronCores (Bass/Tile kernel).

Strategy (per sharding_hint): nodes are row-sharded across the 8 cores
(12500 each); edges are partitioned by dst core so the segment-sum is
local; the small weights are replicated.  h lives in [feat, node]
layout.  Per layer each core all-gathers h in 4 source-quarter
sub-AllGathers (pipelined with compute); source windows (quarters,
25600 nodes) are DMA'd into SBUF and edge-source columns are gathered
in-SBUF with the stock ap_gather GPSIMD instruction (the runtime image
lacks the custom dma_gather ucode; ap_gather measures ~26ns/row, which
is the kernel's bottleneck).  Gathered message columns are transposed
128-at-a-time on the TensorEngine and aggregated with host-prebuilt
one-hot*weight "S" blocks as dense [K<=128 x 256 dst] matmuls into
PSUM: s = 0.9*Ahat@h + 0.1*h0 accumulates into a DRAM tile
[128 feat x nodes] via DMA-staged adds (SBUF is spent on the window);
the layer update h+ = relu(s @ W_eff), W_eff = (1-b)I + b*W, is fused
into one matmul with W_eff precomputed on host.

Everything irregular (degrees, edge normalization, sorting edges into
uniform per-(window, dst-block) cells, gather index / S-block streams)
is precomputed on the host in numpy; the device program is
straight-line fp32 and identical on all 8 cores (SPMD).
"""

import math
import os

import numpy as np

# ---------------- problem constants (full size, hardcoded) ----------------
N = 100000
E = 1600000
IN_DIM = 256
HID = 128
LAYERS = 8
ALPHA = 0.1
THETA = 0.5
NCORES = 8

P = 128          # partitions
NW = 4           # source windows per layer (quarters)
NI = 3072        # gather slots per ap_gather call
CW = 256         # aggregation cell dst width
YB = 512         # output block width

LAST_EXEC_NS = None


class Cfg:
    def __init__(self, n, e, in_dim, hid, layers):
        assert n % NCORES == 0
        self.n, self.e, self.in_dim, self.hid, self.layers = n, e, in_dim, hid, layers
        self.n_per = n // NCORES
        self.n_pad = ((self.n_per + P - 1) // P) * P
        # source quarters: q0..q2 of size qs (multiple of 128), q3 remainder
        qs = ((self.n_per + 3) // 4 + P - 1) // P * P
        self.qs = qs
        q3 = self.n_per - 3 * qs
        assert 0 < q3 <= qs, (self.n_per, qs, q3)
        self.qsizes = [qs, qs, qs, q3]          # real rows per rank-quarter
        for sz in self.qsizes:
            assert 8 * sz <= 32767, "window slot must fit int16"
            assert 8 * sz * 1 <= 2 ** 15, "ap_gather num_elems limit"
        self.NB = self.n_pad // P               # dst 128-blocks per core
        self.NC2 = self.n_pad // CW             # dst cell blocks per core
        assert 3 * qs % P == 0
        self.betas = [float(np.log(THETA / (i + 1) + 1.0)) for i in range(layers)]

    def wsize(self, w):
        """window w (= quarter) node count"""
        return 8 * self.qsizes[w]


def _cfg_full():
    return Cfg(N, E, IN_DIM, HID, LAYERS)


# ---------------- host preprocessing ----------------

def preprocess(x, edge_index, W_in, b_in, convs_W, W_out, b_out, cfg):
    """Build per-core input maps + shared structure metadata."""
    n, n_per, qs = cfg.n, cfg.n_per, cfg.qs
    qsz = np.asarray(cfg.qsizes, np.int64)
    row = np.asarray(edge_index[0], np.int64)
    col = np.asarray(edge_index[1], np.int64)

    deg = np.bincount(col, minlength=n).astype(np.float32) + 1.0
    dinv = (1.0 / np.sqrt(deg)).astype(np.float32)
    wt = ((1.0 - ALPHA) * dinv[row] * dinv[col]).astype(np.float32)

    # append self loops as explicit edges
    allv = np.arange(n, dtype=np.int64)
    row_a = np.concatenate([row, allv])
    col_a = np.concatenate([col, allv])
    wt_a = np.concatenate([wt, ((1.0 - ALPHA) * dinv * dinv).astype(np.float32)])

    # source window + slot within window
    r_s = row_a // n_per
    i_s = row_a % n_per
    q_s = np.minimum(i_s // qs, 3)
    w_s = q_s
    slot = (r_s * qsz[q_s] + (i_s - q_s * qs)).astype(np.int64)

    r_d = col_a // n_per
    dloc = col_a % n_per
    b_idx = dloc // CW                           # dst cell block
    dcol = (dloc % CW).astype(np.int64)

    NB = cfg.NC2
    counts = np.zeros((NCORES, NW, NB), np.int64)
    np.add.at(counts, (r_d, w_s, b_idx), 1)
    n_cb = counts.max(axis=0)                    # [NW, NB]
    n_cb = np.maximum(32 * ((n_cb + 31) // 32), 32)
    L = n_cb.sum(axis=1)
    n_cb[:, NB - 1] += (-L) % P                  # chunk streams multiple of 128
    L = n_cb.sum(axis=1)                         # [NW]

    offs = np.zeros((NW, NB + 1), np.int64)
    offs[:, 1:] = np.cumsum(n_cb, axis=1)

    key = (r_d * NW + w_s) * NB + b_idx
    order = np.argsort(key, kind="stable")
    sk = key[order]
    grp_first = np.r_[0, np.flatnonzero(np.diff(sk)) + 1]
    grp_id = np.zeros(len(sk), np.int64)
    grp_id[grp_first[1:]] = 1
    grp_id = np.cumsum(grp_id)
    rank_in_cell = np.arange(len(sk)) - grp_first[grp_id]
    pos = offs[w_s[order], b_idx[order]] + rank_in_cell

    in_maps = []
    for r in range(NCORES):
        m = {}
        xs = np.zeros((cfg.n_pad, cfg.in_dim), np.float32)
        xs[:n_per] = np.asarray(x[r * n_per:(r + 1) * n_per], np.float32)
        m["x"] = xs
        sel_r = r_d[order] == r
        for c in range(NW):
            sel = sel_r & (w_s[order] == c)
            p = pos[sel]
            idx_stream = np.zeros(L[c], np.int16)
            idx_stream[p] = slot[order][sel].astype(np.int16)
            sarr = np.zeros((P, (L[c] // P) * CW), np.float32)
            sarr[p % P, (p // P) * CW + dcol[order][sel]] = wt_a[order][sel]
            idxd = np.tile(idx_stream.reshape(-1, 16).T, (8, 1))
            m[f"idx{c}"] = np.ascontiguousarray(idxd)
            m[f"sblk{c}"] = sarr
        m["w_in"] = np.asarray(W_in, np.float32)
        m["b_in"] = np.asarray(b_in, np.float32).reshape(cfg.hid, 1)
        weff = np.concatenate(
            [((1.0 - cfg.betas[i]) * np.eye(cfg.hid, dtype=np.float32)
              + cfg.betas[i] * np.asarray(convs_W[i], np.float32))
             for i in range(cfg.layers)], axis=1)
        m["w_eff"] = weff
        m["w_out"] = np.asarray(W_out, np.float32).reshape(cfg.hid, 1)
        m["b_out"] = np.asarray(b_out, np.float32).reshape(1, 1)
        in_maps.append(m)

    return in_maps, {"n_cb": n_cb, "L": L}


# ---------------- device program ----------------

def build(cfg, meta, debug=False):
    import concourse.bass as bass
    import concourse.mybir as mybir
    from concourse import bacc
    from concourse.masks import make_identity
    from concourse.tile import TileContext
    from contextlib import ExitStack

    f32 = mybir.dt.float32
    i16 = mybir.dt.int16
    Relu = mybir.ActivationFunctionType.Relu
    n_cb, L = meta["n_cb"], meta["L"]
    hid, in_dim = cfg.hid, cfg.in_dim
    NB = cfg.NB
    qsz = cfg.qsizes
    SB = 8                   # S groups per stream tile

    nc = bacc.Bacc("TRN2", target_bir_lowering=False, debug=debug)

    x_in = nc.dram_tensor("x", [cfg.n_pad, in_dim], f32, kind="ExternalInput")
    idx_in, s_in = [], []
    for c in range(NW):
        idx_in.append(nc.dram_tensor(f"idx{c}", [P, int(L[c]) // 16], i16,
                                     kind="ExternalInput"))
        s_in.append(nc.dram_tensor(f"sblk{c}", [P, (int(L[c]) // P) * CW], f32,
                                   kind="ExternalInput"))
    w_in_t = nc.dram_tensor("w_in", [in_dim, hid], f32, kind="ExternalInput")
    b_in_t = nc.dram_tensor("b_in", [hid, 1], f32, kind="ExternalInput")
    w_eff_t = nc.dram_tensor("w_eff", [hid, cfg.layers * hid], f32,
                             kind="ExternalInput")
    w_out_t = nc.dram_tensor("w_out", [hid, 1], f32, kind="ExternalInput")
    b_out_t = nc.dram_tensor("b_out", [1, 1], f32, kind="ExternalInput")
    y_out = nc.dram_tensor("y", [1, cfg.n_pad], f32, kind="ExternalOutput")

    rg = [list(range(NCORES))]

    with TileContext(nc) as tc, ExitStack() as ctx:
        const = ctx.enter_context(tc.tile_pool(name="const", bufs=1))
        sfp = ctx.enter_context(tc.tile_pool(name="sfp", bufs=2))
        winp = ctx.enter_context(tc.tile_pool(name="winp", bufs=1))
        gathp = ctx.enter_context(tc.tile_pool(name="gath", bufs=2))
        mtp = ctx.enter_context(tc.tile_pool(name="mtp", bufs=28))
        sblkp = ctx.enter_context(tc.tile_pool(name="sblk", bufs=4))
        idxp = ctx.enter_context(tc.tile_pool(name="idxt", bufs=3))
        xiop = ctx.enter_context(tc.tile_pool(name="xio", bufs=2))
        xtp = ctx.enter_context(tc.tile_pool(name="xt", bufs=3))
        wbp = ctx.enter_context(tc.tile_pool(name="wb", bufs=3))
        h0sp = ctx.enter_context(tc.tile_pool(name="h0sp", bufs=3))
        ytp = ctx.enter_context(tc.tile_pool(name="yt", bufs=2))
        pagg = ctx.enter_context(tc.tile_pool(name="pagg", bufs=3, space="PSUM"))
        ptr = ctx.enter_context(tc.tile_pool(name="ptr", bufs=3, space="PSUM"))
        pmisc = ctx.enter_context(tc.tile_pool(name="pmisc", bufs=2, space="PSUM"))
        dram = ctx.enter_context(tc.tile_pool(name="dram", bufs=1, space="DRAM"))

        # per-quarter h shard in [feat, node] layout + AG'd tables
        h_shard = []
        h_table = [None] * 4
        for q in range(4):
            h_shard.append(dram.tile([P, qsz[q]], f32, tag=f"h_shard{q}",
                                     name=f"h_shard{q}"))
        h0s_dram = dram.tile([P, cfg.n_pad], f32, tag="h0s", name="h0s_dram")

        id1 = const.tile([P, P], f32, tag="id1", name="id1")
        make_identity(nc, id1[:])
        w_in_sb = const.tile([P, (in_dim // P) * hid], f32, tag="w_in",
                             name="w_in_sb")
        for k in range(in_dim // P):
            nc.sync.dma_start(out=w_in_sb[:, k * hid:(k + 1) * hid],
                              in_=w_in_t[k * P:(k + 1) * P, :])
        b_in_sb = const.tile([P, 1], f32, tag="b_in", name="b_in_sb")
        nc.sync.dma_start(out=b_in_sb[:], in_=b_in_t[:])
        b_in_s = const.tile([P, 1], f32, tag="b_in_s", name="b_in_s")
        nc.vector.tensor_scalar_mul(b_in_s[:], b_in_sb[:], ALPHA)
        w_eff_sb = const.tile([P, cfg.layers * hid], f32, tag="w_eff",
                              name="w_eff_sb")
        nc.sync.dma_start(out=w_eff_sb[:], in_=w_eff_t[:])
        w_out_sb = const.tile([P, 1], f32, tag="w_out", name="w_out_sb")
        nc.sync.dma_start(out=w_out_sb[:], in_=w_out_t[:])
        b_out_sb = const.tile([1, 1], f32, tag="b_out", name="b_out_sb")
        nc.sync.dma_start(out=b_out_sb[:], in_=b_out_t[:])

        s_acc = dram.tile([P, cfg.n_pad], f32, tag="s_acc", name="s_acc")

        reg_cache = {}

        def nreg(v):
            if v not in reg_cache:
                reg_cache[v] = nc.gpsimd.to_reg(v)
            return reg_cache[v]

        def emit_ag(q):
            tab = dram.tile([NCORES * P, qsz[q]], f32, tag=f"h_table{q}",
                            name=f"h_table{q}", addr_space="Shared", bufs=2)
            nc.gpsimd.collective_compute(
                "AllGather", mybir.AluOpType.bypass, replica_groups=rg,
                ins=[h_shard[q][:, 0:qsz[q]].opt()],
                outs=[tab[:].opt()])
            h_table[q] = tab

        def shard_cols(blk, width):
            """node block (width cols from blk*width) -> (quarter, col off)"""
            lo = blk * width
            q = min(lo // cfg.qs, 3)
            return q, lo - q * cfg.qs

        # ---------------- init: h0 = relu(x@W_in + b_in) ----------------
        for nt in range(NB):
            x_tile = xiop.tile([P, in_dim], f32, tag="x", name="x_tile")
            nc.sync.dma_start(out=x_tile[:], in_=x_in[nt * P:(nt + 1) * P, :])
            xts = []
            for k in range(in_dim // P):
                xt_ps = ptr.tile([P, P], f32, tag="ptr", name="xt_ps")
                nc.tensor.transpose(xt_ps[:], x_tile[:, k * P:(k + 1) * P], id1[:])
                xt_sb = xtp.tile([P, P], f32, tag="xt", name="xt_sb")
                nc.vector.tensor_copy(out=xt_sb[:], in_=xt_ps[:])
                xts.append(xt_sb)
            ph0 = pmisc.tile([P, YB], f32, tag="pmisc", name="pm")
            nk = in_dim // P
            for k in range(nk):
                nc.tensor.matmul(ph0[:, :P], lhsT=w_in_sb[:, k * hid:(k + 1) * hid],
                                 rhs=xts[k][:], start=(k == 0), stop=(k == nk - 1))
            wb = wbp.tile([P, YB], f32, tag="wb", name="wb")
            nc.scalar.activation(wb[:, :P], ph0[:, :P], Relu, bias=b_in_sb[:])
            q, co = shard_cols(nt, P)
            take = min(P, qsz[q] - co)
            nc.sync.dma_start(out=h_shard[q][:, co:co + take], in_=wb[:, :take])
            h0t = h0sp.tile([P, P], f32, tag="h0t", name="h0t")
            nc.scalar.activation(h0t[:], ph0[:, :P], Relu, bias=b_in_s[:],
                                 scale=ALPHA)
            nc.sync.dma_start(out=h0s_dram[:, nt * P:(nt + 1) * P], in_=h0t[:])
            if q < 3 and nt == (q + 1) * (cfg.qs // P) - 1:
                emit_ag(q)
            elif nt == NB - 1:
                emit_ag(3)

        # ---------------- layers ----------------
        for layer in range(cfg.layers):
            last = layer == cfg.layers - 1
            tables = list(h_table)

            def finish_block(b):
                """512-wide output block b of s_acc is complete"""
                w = min(YB, cfg.n_pad - b * YB)
                cols = slice(b * YB, b * YB + w)
                sf = sfp.tile([P, YB], f32, tag="sf", name="sf")
                nc.sync.dma_start(out=sf[:, :w], in_=s_acc[:, cols])
                ps = pmisc.tile([P, YB], f32, tag="pmisc", name="pm")
                nc.tensor.matmul(ps[:, :w],
                                 lhsT=w_eff_sb[:, layer * hid:(layer + 1) * hid],
                                 rhs=sf[:, :w], start=True, stop=True)
                if not last:
                    wb = wbp.tile([P, YB], f32, tag="wb", name="wb")
                    nc.scalar.activation(wb[:, :w], ps[:, :w], Relu)
                    done = 0
                    while done < w:       # may straddle quarter boundary
                        q = min((b * YB + done) // cfg.qs, 3)
                        co = b * YB + done - q * cfg.qs
                        take = min(w - done, qsz[q] - co)
                        if take <= 0:     # pad columns past the real nodes
                            break
                        nc.sync.dma_start(out=h_shard[q][:, co:co + take],
                                          in_=wb[:, done:done + take])
                        done += take
                    # fire AGs when a quarter's columns are all written
                    hi = b * YB + w
                    for q in range(3):
                        if b * YB < (q + 1) * cfg.qs <= hi:
                            emit_ag(q)
                    if hi == cfg.n_pad:
                        emit_ag(3)
                else:
                    h8 = wbp.tile([P, YB], f32, tag="wb", name="wb")
                    nc.scalar.activation(h8[:, :w], ps[:, :w], Relu)
                    psy = pmisc.tile([P, YB], f32, tag="pmisc", name="pm")
                    nc.tensor.matmul(psy[0:1, :w], lhsT=w_out_sb[:, 0:1],
                                     rhs=h8[:, :w], start=True, stop=True)
                    yt = ytp.tile([1, YB], f32, tag="yt", name="yt")
                    nc.vector.tensor_tensor(
                        out=yt[0:1, :w], in0=psy[0:1, :w],
                        in1=b_out_sb[0:1, 0:1].to_broadcast([1, w]),
                        op=mybir.AluOpType.add)
                    nc.sync.dma_start(out=y_out[0:1, b * YB:b * YB + w],
                                      in_=yt[0:1, :w])

            for c in range(NW):
                q = c
                wsz = cfg.wsize(c)
                # load window: all 8 rank blocks of the AG'd quarter table
                win = winp.tile([P, 8 * cfg.qs], f32, tag="win", name="win")
                for rr in range(8):
                    nc.sync.dma_start(
                        out=win[:, rr * qsz[q]:(rr + 1) * qsz[q]],
                        in_=tables[q][rr * P:(rr + 1) * P, :])
                win3 = win[:, :wsz].rearrange("p (n d) -> p n d", d=1)

                Lc = int(L[c])
                nsg = (Lc + NI - 1) // NI
                mt_tiles = {}          # 128-slot group -> sbuf [slot, feat]
                s_tiles = [None] * ((Lc // P + SB - 1) // SB)

                def ensure_group(g, c=c, win3=win3, Lc=Lc, mt_tiles=mt_tiles,
                                 s_tiles=s_tiles):
                    """gather+transpose 128-slot group g; returns sbuf tiles"""
                    if g in mt_tiles:
                        return
                    sg = (g * P) // NI
                    slots = min(NI, Lc - sg * NI)
                    gt = gathp.tile([P, NI], f32, tag="gt", name="gt")
                    it = idxp.tile([P, NI // 16], i16, tag="it", name="it")
                    nc.sync.dma_start(
                        out=it[:, :slots // 16],
                        in_=idx_in[c][:, sg * (NI // 16):sg * (NI // 16) + slots // 16])
                    nc.gpsimd.ap_gather(
                        gt[:, :slots].rearrange("p (n d) -> p n d", d=1), win3,
                        it[:, :slots // 16], P, wsz, 1, slots)
                    g_lo = sg * (NI // P)
                    g_hi = g_lo + slots // P - 1
                    for gg in range(g_lo, g_hi + 1):
                        ps = ptr.tile([P, P], f32, tag="ptr", name="tr_ps")
                        off = gg * P - sg * NI
                        nc.tensor.transpose(ps[:], gt[:, off:off + P], id1[:])
                        mt = mtp.tile([P, P], f32, tag="mt", name="mt")
                        nc.vector.tensor_copy(out=mt[:], in_=ps[:])
                        mt_tiles[gg] = mt
                    for sb in range(g_lo // SB, min(g_hi // SB + 2,
                                                    len(s_tiles))):
                        if s_tiles[sb] is None:
                            st = sblkp.tile([P, SB * CW], f32, tag="st", name="st")
                            lo = sb * SB * CW
                            ncols = min(SB * CW, (Lc // P) * CW - lo)
                            nc.sync.dma_start(out=st[:, :ncols],
                                              in_=s_in[c][:, lo:lo + ncols])
                            s_tiles[sb] = st

                cur = 0
                for b in range(cfg.NC2):
                    ps_b = pagg.tile([P, CW], f32, tag="ps_b", name="ps_b")
                    n_slots = int(n_cb[c][b])
                    first = True
                    left = n_slots
                    while left > 0:
                        g, p0 = cur // P, cur % P
                        ln = 0
                        for sz in (128, 64, 32):
                            if p0 % sz == 0 and left >= sz and p0 + sz <= P:
                                ln = sz
                                break
                        assert ln, (p0, left)
                        ensure_group(g)
                        st = s_tiles[g // SB]
                        so = (g % SB) * CW
                        nc.tensor.matmul(
                            ps_b[:],
                            lhsT=mt_tiles[g][p0:p0 + ln, :],
                            rhs=st[p0:p0 + ln, so:so + CW],
                            start=first, stop=(ln == left),
                            tile_position=(p0, 0))
                        first = False
                        cur += ln
                        left -= ln
                    sa = h0sp.tile([P, CW], f32, tag="h0t", name="sa")
                    src_t = h0s_dram if c == 0 else s_acc
                    nc.sync.dma_start(out=sa[:],
                                      in_=src_t[:, b * CW:(b + 1) * CW])
                    nc.vector.tensor_add(out=sa[:], in0=sa[:], in1=ps_b[:])
                    nc.sync.dma_start(out=s_acc[:, b * CW:(b + 1) * CW],
                                      in_=sa[:])
                    if c == NW - 1 and (((b + 1) * CW) % YB == 0 or
                                        b == cfg.NC2 - 1):
                        finish_block((b * CW) // YB)

    nc.compile()
    return nc


# ---------------- top level ----------------

def _assemble_y(results, cfg):
    parts = []
    for r in range(NCORES):
        y = np.asarray(results[r]["y"], np.float32).reshape(-1)
        parts.append(y[:cfg.n_per])
    return np.concatenate(parts)


def _run_pjrt(nc, in_maps, n_cores, time_iters=0):
    """Execute the bass program on the NeuronCores via PJRT (the axon
    redirect path of run_bass_kernel_spmd), with inputs pre-staged on
    device.  Mirrors concourse.bass2jax.run_bass_via_pjrt (multi-core).

    The axon dispatch floor is ~80ms/call, so single-call wall time says
    nothing about device time; with time_iters > 0 the marginal cost per
    execute between pipelined batches of M_lo and M_hi back-to-back
    calls is reported: device exec time plus ~1ms per-call dispatch (an
    honest upper bound on HW time).
    """
    import time
    import jax
    from jax.sharding import Mesh, NamedSharding, PartitionSpec
    from jax.experimental.shard_map import shard_map
    from concourse import bass2jax, mybir

    bass2jax.install_neuronx_cc_hook()

    partition_name = nc.partition_id_tensor.name if nc.partition_id_tensor else None
    in_names, out_names, out_avals, zero_outs = [], [], [], []
    for alloc in nc.m.functions[0].allocations:
        if not isinstance(alloc, mybir.MemoryLocationSet):
            continue
        name = alloc.memorylocations[0].name
        if alloc.kind == "ExternalInput":
            if name != partition_name:
                in_names.append(name)
        elif alloc.kind == "ExternalOutput":
            out_names.append(name)
            shape = tuple(alloc.tensor_shape)
            dtype = mybir.dt.np(alloc.dtype)
            out_avals.append(jax.core.ShapedArray(shape, dtype))
            zero_outs.append(np.zeros(shape, dtype))
    n_params = len(in_names)
    n_outs = len(out_avals)
    in_names.extend(out_names)
    if partition_name is not None:
        in_names.append(partition_name)
    donate = tuple(range(n_params, n_params + n_outs))

    def _body(*args):
        operands = list(args)
        if partition_name is not None:
            operands.append(bass2jax.partition_id_tensor())
        outs = bass2jax._bass_exec_p.bind(
            *operands,
            out_avals=tuple(out_avals),
            in_names=tuple(in_names),
            out_names=tuple(out_names),
            lowering_input_output_aliases=(),
            sim_require_finite=True,
            sim_require_nnan=True,
            nc=nc,
        )
        return tuple(outs)

    devices = jax.devices()[:n_cores]
    assert len(devices) == n_cores
    mesh = Mesh(np.asarray(devices), ("core",))
    in_specs = (PartitionSpec("core"),) * (n_params + n_outs)
    out_specs = (PartitionSpec("core"),) * len(out_names)
    sharded = jax.jit(
        shard_map(_body, mesh=mesh, in_specs=in_specs, out_specs=out_specs,
                  check_rep=False),
        donate_argnums=donate, keep_unused=True)

    shard = NamedSharding(mesh, PartitionSpec("core"))
    concat_in = [
        jax.device_put(
            np.concatenate([np.asarray(in_maps[c][name]) for c in range(n_cores)],
                           axis=0), shard)
        for name in in_names[:n_params]
    ]
    jax.block_until_ready(concat_in)

    def zeros():
        return [
            jax.device_put(np.zeros((n_cores * z.shape[0], *z.shape[1:]), z.dtype),
                           shard)
            for z in zero_outs
        ]

    out_arrs = jax.block_until_ready(sharded(*concat_in, *zeros()))
    exec_ns = None
    if time_iters > 0:
        m_lo, m_hi = 4, 4 + max(4, time_iters)

        def run_m(m):
            zs = [zeros() for _ in range(m)]
            jax.block_until_ready(zs)
            t0 = time.perf_counter()
            rs = [sharded(*concat_in, *z) for z in zs]
            jax.block_until_ready(rs)
            return time.perf_counter() - t0

        run_m(2)  # warm
        lo = min(run_m(m_lo) for _ in range(2))
        hi = min(run_m(m_hi) for _ in range(2))
        exec_ns = int(max(hi - lo, 0) / (m_hi - m_lo) * 1e9)
    results = [
        {name: np.asarray(out_arrs[i]).reshape(n_cores, *out_avals[i].shape)[c]
         for i, name in enumerate(out_names)}
        for c in range(n_cores)
    ]
    return results, exec_ns


def kernel(x, edge_index, W_in, b_in, convs_W, W_out, b_out):
    global LAST_EXEC_NS
    cfg = _cfg_full()
    in_maps, meta = preprocess(x, edge_index, W_in, b_in, convs_W, W_out, b_out,
                               cfg)
    nc = build(cfg, meta)
    iters = int(os.environ.get("KERNEL_TIME_ITERS", "0"))
    results, exec_ns = _run_pjrt(nc, in_maps, NCORES, time_iters=iters)
    LAST_EXEC_NS = exec_ns
    return _assemble_y(results, cfg)



# revision 7
# speedup vs baseline: 2.0817x; 2.0817x over previous
"""GCNII node regressor on 8 trn2 NeuronCores (Bass/Tile kernel), v2.

Strategy: nodes row-sharded across 8 cores (12500 each); edges partitioned
by dst core so the segment-sum is local; small weights replicated.

v2 replaces the ap_gather (GPSIMD, ~26ns/edge) message gather of v1 with
the stock indirect DMA (gpsimd indirect_dma_start -> dma_memcopy_indirect
ucode): h lives NODE-major ([node, 128 feat] fp16, 256B rows) in per-core
DRAM tables (AllGather'd per source quarter), and each slot chunk is
gathered straight into [slot%128 partition, slot//128, feat] SBUF layout
by the DMA engines -- also eliminating the per-group PE transposes and
the 100KB/partition SBUF window of v1.  The scatter side keeps v1's
host-prebuilt one-hot*weight "S" blocks, now fp16 and CW=128 dst columns
per cell: s = 0.9*Ahat@h + 0.1*h0 accumulates in PSUM per cell and lands
in an SBUF-resident s accumulator (no DRAM staging).  The layer update
h+ = relu(s @ W_eff) with W_eff = (1-b)I + b*W runs fp32 from s_sb; the
fp16 result is PE-transposed back to node-major and written to the next
h shard, with per-quarter AllGathers pipelined into the finish phase.

Everything irregular (degrees, normalization, edge sorting into uniform
per-(window, dst-cell) streams, gather index / S-block streams) is
precomputed on the host in numpy; the device program is straight-line and
identical on all 8 cores (SPMD).
"""

import math
import os

import numpy as np
import ml_dtypes

# ---------------- problem constants (full size, hardcoded) ----------------
N = 100000
E = 1600000
IN_DIM = 256
HID = 128
LAYERS = 8
ALPHA = 0.1
THETA = 0.5
NCORES = 8

P = 128          # partitions
NW = 4           # source windows per layer (quarters)
NI = 12288       # gather slots per dma_gather call
CW = 128         # aggregation cell dst width
YB = 512         # output block width
SB = 8           # S groups per stream tile

H_DT = np.float16         # h table / gather / S dtype

LAST_EXEC_NS = None


class Cfg:
    def __init__(self, n, e, in_dim, hid, layers):
        assert n % NCORES == 0
        self.n, self.e, self.in_dim, self.hid, self.layers = n, e, in_dim, hid, layers
        self.n_per = n // NCORES
        self.n_pad = ((self.n_per + P - 1) // P) * P
        # source quarters: q0..q2 of size qs (multiple of 128), q3 remainder
        qs = ((self.n_per + 3) // 4 + P - 1) // P * P
        self.qs = qs
        self.n_pad = ((self.n_per + P - 1) // P) * P
        q3 = self.n_pad - 3 * qs                # q3 padded so quarters tile n_pad
        assert 0 < q3 <= qs and q3 % P == 0, (self.n_per, qs, q3)
        self.qsizes = [qs, qs, qs, q3]          # rows per rank-quarter (q3 padded)
        for sz in self.qsizes:
            assert 8 * sz <= 32767, "table row index must fit int16"
        self.NB = self.n_pad // P               # dst 128-blocks per core
        self.NC2 = self.n_pad // CW             # dst cell blocks per core
        self.betas = [float(np.log(THETA / (i + 1) + 1.0)) for i in range(layers)]

    def wsize(self, w):
        """window w (= quarter) node count (table rows)"""
        return 8 * self.qsizes[w]


def _cfg_full():
    return Cfg(N, E, IN_DIM, HID, LAYERS)


# ---------------- host preprocessing ----------------

def preprocess(x, edge_index, W_in, b_in, convs_W, W_out, b_out, cfg):
    """Build per-core input maps + shared structure metadata."""
    n, n_per, qs = cfg.n, cfg.n_per, cfg.qs
    qsz = np.asarray(cfg.qsizes, np.int64)
    row = np.asarray(edge_index[0], np.int64)
    col = np.asarray(edge_index[1], np.int64)

    deg = np.bincount(col, minlength=n).astype(np.float32) + 1.0
    dinv = (1.0 / np.sqrt(deg)).astype(np.float32)
    wt = ((1.0 - ALPHA) * dinv[row] * dinv[col]).astype(np.float32)

    # append self loops as explicit edges
    allv = np.arange(n, dtype=np.int64)
    row_a = np.concatenate([row, allv])
    col_a = np.concatenate([col, allv])
    wt_a = np.concatenate([wt, ((1.0 - ALPHA) * dinv * dinv).astype(np.float32)])

    # source window (quarter) + table row within window
    r_s = row_a // n_per
    i_s = row_a % n_per
    q_s = np.minimum(i_s // qs, 3)
    w_s = q_s
    slot = (r_s * qsz[q_s] + (i_s - q_s * qs)).astype(np.int64)

    r_d = col_a // n_per
    dloc = col_a % n_per
    b_idx = dloc // CW                           # dst cell block
    dcol = (dloc % CW).astype(np.int64)

    NB = cfg.NC2
    counts = np.zeros((NCORES, NW, NB), np.int64)
    np.add.at(counts, (r_d, w_s, b_idx), 1)
    n_cb = counts.max(axis=0)                    # [NW, NB]
    n_cb = np.maximum(32 * ((n_cb + 31) // 32), 32)
    L = n_cb.sum(axis=1)
    n_cb[:, NB - 1] += (-L) % P                  # slot streams multiple of 128
    L = n_cb.sum(axis=1)                         # [NW]

    offs = np.zeros((NW, NB + 1), np.int64)
    offs[:, 1:] = np.cumsum(n_cb, axis=1)

    key = (r_d * NW + w_s) * NB + b_idx
    order = np.argsort(key, kind="stable")
    sk = key[order]
    grp_first = np.r_[0, np.flatnonzero(np.diff(sk)) + 1]
    grp_id = np.zeros(len(sk), np.int64)
    grp_id[grp_first[1:]] = 1
    grp_id = np.cumsum(grp_id)
    rank_in_cell = np.arange(len(sk)) - grp_first[grp_id]
    pos = offs[w_s[order], b_idx[order]] + rank_in_cell

    in_maps = []
    for r in range(NCORES):
        m = {}
        xs = np.zeros((cfg.n_pad, cfg.in_dim), np.float32)
        xs[:n_per] = np.asarray(x[r * n_per:(r + 1) * n_per], np.float32)
        m["x"] = xs
        sel_r = r_d[order] == r
        for c in range(NW):
            sel = sel_r & (w_s[order] == c)
            p = pos[sel]
            idx_arr = np.zeros((P, int(L[c]) // P), np.int32)
            idx_arr[p % P, p // P] = slot[order][sel].astype(np.int32)
            sarr = np.zeros((P, (L[c] // P) * CW), H_DT)
            sarr[p % P, (p // P) * CW + dcol[order][sel]] = wt_a[order][sel]
            m[f"idx{c}"] = idx_arr
            m[f"sblk{c}"] = sarr
        m["w_in"] = np.asarray(W_in, np.float32)
        m["b_in"] = np.asarray(b_in, np.float32).reshape(cfg.hid, 1)
        weff = np.concatenate(
            [((1.0 - cfg.betas[i]) * np.eye(cfg.hid, dtype=np.float32)
              + cfg.betas[i] * np.asarray(convs_W[i], np.float32))
             for i in range(cfg.layers)], axis=1)
        m["w_eff"] = weff
        m["w_out"] = np.asarray(W_out, np.float32).reshape(cfg.hid, 1)
        m["b_out"] = np.asarray(b_out, np.float32).reshape(1, 1)
        in_maps.append(m)

    return in_maps, {"n_cb": n_cb, "L": L}


# ---------------- device program ----------------

def build(cfg, meta, debug=False):
    import concourse.bass as bass
    import concourse.mybir as mybir
    from concourse import bacc
    from concourse.masks import make_identity
    from concourse.tile import TileContext
    from contextlib import ExitStack

    f32 = mybir.dt.float32
    f16 = mybir.dt.float16
    i32 = mybir.dt.int32
    Relu = mybir.ActivationFunctionType.Relu
    n_cb, L = meta["n_cb"], meta["L"]
    hid, in_dim = cfg.hid, cfg.in_dim
    qsz = cfg.qsizes

    nc = bacc.Bacc("TRN2", target_bir_lowering=False, debug=debug)

    x_in = nc.dram_tensor("x", [cfg.n_pad, in_dim], f32, kind="ExternalInput")
    idx_in, s_in = [], []
    for c in range(NW):
        idx_in.append(nc.dram_tensor(f"idx{c}", [P, int(L[c]) // P], i32,
                                     kind="ExternalInput"))
        s_in.append(nc.dram_tensor(f"sblk{c}", [P, (int(L[c]) // P) * CW], f16,
                                   kind="ExternalInput"))
    w_in_t = nc.dram_tensor("w_in", [in_dim, hid], f32, kind="ExternalInput")
    b_in_t = nc.dram_tensor("b_in", [hid, 1], f32, kind="ExternalInput")
    w_eff_t = nc.dram_tensor("w_eff", [hid, cfg.layers * hid], f32,
                             kind="ExternalInput")
    w_out_t = nc.dram_tensor("w_out", [hid, 1], f32, kind="ExternalInput")
    b_out_t = nc.dram_tensor("b_out", [1, 1], f32, kind="ExternalInput")
    y_out = nc.dram_tensor("y", [1, cfg.n_pad], f32, kind="ExternalOutput")

    rg = [list(range(NCORES))]

    with TileContext(nc) as tc, ExitStack() as ctx:
        const = ctx.enter_context(tc.tile_pool(name="const", bufs=1))
        gathp = ctx.enter_context(tc.tile_pool(name="gath", bufs=32))
        sblkp = ctx.enter_context(tc.tile_pool(name="sblk", bufs=4))
        idxp = ctx.enter_context(tc.tile_pool(name="idxt", bufs=2))
        xiop = ctx.enter_context(tc.tile_pool(name="xio", bufs=2))
        xtp = ctx.enter_context(tc.tile_pool(name="xt", bufs=3))
        wbp = ctx.enter_context(tc.tile_pool(name="wb", bufs=3))
        hbp = ctx.enter_context(tc.tile_pool(name="hb", bufs=2))
        h4p = ctx.enter_context(tc.tile_pool(name="h4", bufs=2))
        ytp = ctx.enter_context(tc.tile_pool(name="yt", bufs=2))
        pagg = ctx.enter_context(tc.tile_pool(name="pagg", bufs=3, space="PSUM"))
        ptr = ctx.enter_context(tc.tile_pool(name="ptr", bufs=3, space="PSUM"))
        pmisc = ctx.enter_context(tc.tile_pool(name="pmisc", bufs=2, space="PSUM"))
        dram = ctx.enter_context(tc.tile_pool(name="dram", bufs=1, space="DRAM"))

        # node-major per-quarter h shard + AG'd tables (fp16)
        h_shard = []
        h_table = [None] * 4
        for q in range(4):
            h_shard.append(dram.tile([qsz[q], P], f16, tag=f"h_shard{q}",
                                     name=f"h_shard{q}"))

        id_f = const.tile([P, P], f32, tag="id_f", name="id_f")
        make_identity(nc, id_f[:])
        w_in_sb = const.tile([P, (in_dim // P) * hid], f32, tag="w_in",
                             name="w_in_sb")
        for k in range(in_dim // P):
            nc.sync.dma_start(out=w_in_sb[:, k * hid:(k + 1) * hid],
                              in_=w_in_t[k * P:(k + 1) * P, :])
        b_in_sb = const.tile([P, 1], f32, tag="b_in", name="b_in_sb")
        nc.sync.dma_start(out=b_in_sb[:], in_=b_in_t[:])
        b_in_s = const.tile([P, 1], f32, tag="b_in_s", name="b_in_s")
        nc.vector.tensor_scalar_mul(b_in_s[:], b_in_sb[:], ALPHA)
        w_eff_sb = const.tile([P, cfg.layers * hid], f32, tag="w_eff",
                              name="w_eff_sb")
        nc.sync.dma_start(out=w_eff_sb[:], in_=w_eff_t[:])
        w_out_sb = const.tile([P, 1], f32, tag="w_out", name="w_out_sb")
        nc.sync.dma_start(out=w_out_sb[:], in_=w_out_t[:])
        b_out_sb = const.tile([1, 1], f32, tag="b_out", name="b_out_sb")
        nc.sync.dma_start(out=b_out_sb[:], in_=b_out_t[:])

        # SBUF-resident accumulators
        s_sb = const.tile([P, cfg.n_pad], f32, tag="s_sb", name="s_sb")
        h0a_sb = const.tile([P, cfg.n_pad], f32, tag="h0a", name="h0a_sb")

        def emit_ag(q):
            tab = dram.tile([NCORES * qsz[q], P], f16, tag=f"h_table{q}",
                            name=f"h_table{q}", addr_space="Shared", bufs=2)
            nc.gpsimd.collective_compute(
                "AllGather", mybir.AluOpType.bypass, replica_groups=rg,
                ins=[h_shard[q][:, :].opt()],
                outs=[tab[:].opt()])
            h_table[q] = tab

        def write_h_rows(hs4, nblk, lo):
            """DMA hs4[:, :nblk, :] (node%128, blk, feat) to node-major
            h_shard rows [lo, lo+128*nblk), splitting at quarter bounds."""
            done = 0
            while done < nblk:
                pos = lo + done * P
                q = min(pos // cfg.qs, 3)
                co = pos - q * cfg.qs
                take = min(nblk - done, (qsz[q] - co) // P)
                if take <= 0:
                    break                 # pad rows past the real nodes
                dst = h_shard[q][co:co + take * P, :].rearrange(
                    "(j p) d -> p j d", p=P)
                nc.sync.dma_start(out=dst, in_=hs4[:, done:done + take, :])
                done += take

        def fire_ags(lo, hi):
            for q in range(3):
                if lo < (q + 1) * cfg.qs <= hi:
                    emit_ag(q)
            if hi >= cfg.n_pad:
                emit_ag(3)

        # ---------------- init: h0 = relu(x@W_in + b_in) ----------------
        nblk_grp = 4
        for nt0 in range(0, cfg.NB, nblk_grp):
            nb = min(nblk_grp, cfg.NB - nt0)
            hs4 = h4p.tile([P, nblk_grp, P], f16, tag="hs4", name="hs4")
            for j in range(nb):
                nt = nt0 + j
                x_tile = xiop.tile([P, in_dim], f32, tag="x", name="x_tile")
                nc.sync.dma_start(out=x_tile[:], in_=x_in[nt * P:(nt + 1) * P, :])
                xts = []
                for k in range(in_dim // P):
                    xt_ps = ptr.tile([P, P], f32, tag="ptr", name="xt_ps")
                    nc.tensor.transpose(xt_ps[:], x_tile[:, k * P:(k + 1) * P],
                                        id_f[:])
                    xt_sb = xtp.tile([P, P], f32, tag="xt", name="xt_sb")
                    nc.vector.tensor_copy(out=xt_sb[:], in_=xt_ps[:])
                    xts.append(xt_sb)
                ph0 = pmisc.tile([P, YB], f32, tag="pmisc", name="pm")
                nk = in_dim // P
                for k in range(nk):
                    nc.tensor.matmul(ph0[:, :P],
                                     lhsT=w_in_sb[:, k * hid:(k + 1) * hid],
                                     rhs=xts[k][:], start=(k == 0),
                                     stop=(k == nk - 1))
                # alpha * h0 stays feat-major in SBUF
                nc.scalar.activation(h0a_sb[:, nt * P:(nt + 1) * P], ph0[:, :P],
                                     Relu, bias=b_in_s[:], scale=ALPHA)
                # h -> transpose to node-major, cast fp16 on copy-out
                hb = wbp.tile([P, P], f32, tag="wb", name="hbi")
                nc.scalar.activation(hb[:], ph0[:, :P], Relu, bias=b_in_sb[:])
                pt = ptr.tile([P, P], f32, tag="ptr", name="pt")
                nc.tensor.transpose(pt[:], hb[:], id_f[:])
                nc.vector.tensor_copy(out=hs4[:, j, :], in_=pt[:])
            write_h_rows(hs4, nb, nt0 * P)
            fire_ags(nt0 * P, (nt0 + nb) * P)

        # ---------------- layers ----------------
        for layer in range(cfg.layers):
            last = layer == cfg.layers - 1
            tables = list(h_table)

            def finish_block(b, tables=tables, layer=layer, last=last):
                """YB-wide output block b of s_sb is complete"""
                w = min(YB, cfg.n_pad - b * YB)
                cols = slice(b * YB, b * YB + w)
                ps = pmisc.tile([P, YB], f32, tag="pmisc", name="pm")
                nc.tensor.matmul(ps[:, :w],
                                 lhsT=w_eff_sb[:, layer * hid:(layer + 1) * hid],
                                 rhs=s_sb[:, cols], start=True, stop=True)
                if not last:
                    hb = hbp.tile([P, YB], f32, tag="hb", name="hb")
                    nc.scalar.activation(hb[:, :w], ps[:, :w], Relu)
                    hs4 = h4p.tile([P, YB // P, P], f16, tag="hs4f", name="hs4f")
                    for j in range(w // P):
                        pt = ptr.tile([P, P], f32, tag="ptr", name="pt")
                        nc.tensor.transpose(pt[:], hb[:, j * P:(j + 1) * P],
                                            id_f[:])
                        nc.vector.tensor_copy(out=hs4[:, j, :], in_=pt[:])
                    write_h_rows(hs4, w // P, b * YB)
                    fire_ags(b * YB, b * YB + w)
                else:
                    h8 = wbp.tile([P, YB], f32, tag="wb", name="wb")
                    nc.scalar.activation(h8[:, :w], ps[:, :w], Relu)
                    psy = pmisc.tile([P, YB], f32, tag="pmisc", name="pm")
                    nc.tensor.matmul(psy[0:1, :w], lhsT=w_out_sb[:, 0:1],
                                     rhs=h8[:, :w], start=True, stop=True)
                    yt = ytp.tile([1, YB], f32, tag="yt", name="yt")
                    nc.vector.tensor_tensor(
                        out=yt[0:1, :w], in0=psy[0:1, :w],
                        in1=b_out_sb[0:1, 0:1].to_broadcast([1, w]),
                        op=mybir.AluOpType.add)
                    nc.sync.dma_start(out=y_out[0:1, b * YB:b * YB + w],
                                      in_=yt[0:1, :w])

            for c in range(NW):
                Lc = int(L[c])
                ng = Lc // P
                it_win = idxp.tile([P, ng], i32, tag=f"itw{c}", name="it_win")
                nc.sync.dma_start(out=it_win[:], in_=idx_in[c][:])
                gt_tiles = {}
                s_tiles = [None] * ((ng + SB - 1) // SB)

                def ensure_group(g, c=c, it_win=it_win, gt_tiles=gt_tiles,
                                 tables=tables):
                    if g in gt_tiles:
                        return
                    gt = gathp.tile([P, P], f16, tag="gt", name="gt")
                    nc.gpsimd.indirect_dma_start(
                        out=gt[:], out_offset=None,
                        in_=tables[c][:],
                        in_offset=bass.IndirectOffsetOnAxis(
                            ap=it_win[:, g:g + 1], axis=0))
                    gt_tiles[g] = gt

                def ensure_s(sb, c=c, ng=ng, s_tiles=s_tiles):
                    if s_tiles[sb] is not None:
                        return
                    st = sblkp.tile([P, SB * CW], f16, tag="st", name="st")
                    lo = sb * SB * CW
                    ncols = min(SB * CW, ng * CW - lo)
                    nc.sync.dma_start(out=st[:, :ncols],
                                      in_=s_in[c][:, lo:lo + ncols])
                    s_tiles[sb] = st

                cur = 0
                for b in range(cfg.NC2):
                    ps_b = pagg.tile([P, CW], f32, tag="ps_b", name="ps_b")
                    n_slots = int(n_cb[c][b])
                    first = True
                    left = n_slots
                    while left > 0:
                        g, p0 = cur // P, cur % P
                        ln = 0
                        for sz in (128, 64, 32):
                            if p0 % sz == 0 and left >= sz and p0 + sz <= P:
                                ln = sz
                                break
                        assert ln, (p0, left)
                        ensure_group(g)
                        ensure_s(g // SB)
                        st = s_tiles[g // SB]
                        so = (g % SB) * CW
                        nc.tensor.matmul(
                            ps_b[:],
                            lhsT=gt_tiles[g][p0:p0 + ln, :],
                            rhs=st[p0:p0 + ln, so:so + CW],
                            start=first, stop=(ln == left),
                            tile_position=(p0, 0))
                        first = False
                        cur += ln
                        left -= ln
                    cols = slice(b * CW, (b + 1) * CW)
                    if c == 0:
                        nc.vector.tensor_add(out=s_sb[:, cols],
                                             in0=h0a_sb[:, cols], in1=ps_b[:])
                    else:
                        nc.vector.tensor_add(out=s_sb[:, cols],
                                             in0=s_sb[:, cols], in1=ps_b[:])
                    if c == NW - 1 and (((b + 1) * CW) % YB == 0 or
                                        b == cfg.NC2 - 1):
                        finish_block((b * CW) // YB)

    nc.compile()
    return nc


# ---------------- top level ----------------

def _assemble_y(results, cfg):
    parts = []
    for r in range(NCORES):
        y = np.asarray(results[r]["y"], np.float32).reshape(-1)
        parts.append(y[:cfg.n_per])
    return np.concatenate(parts)


def _run_pjrt(nc, in_maps, n_cores, time_iters=0, devices=None, donate=True):
    """Execute the bass program on the NeuronCores via PJRT (the axon
    redirect path of run_bass_kernel_spmd), with inputs pre-staged on
    device.  Mirrors concourse.bass2jax.run_bass_via_pjrt (multi-core).

    The axon dispatch floor is ~80ms/call, so single-call wall time says
    nothing about device time; with time_iters > 0 the marginal cost per
    execute between pipelined batches of M_lo and M_hi back-to-back
    calls is reported: device exec time plus ~1ms per-call dispatch (an
    honest upper bound on HW time).
    """
    import time
    import jax
    from jax.sharding import Mesh, NamedSharding, PartitionSpec
    from jax.experimental.shard_map import shard_map
    from concourse import bass2jax, mybir

    bass2jax.install_neuronx_cc_hook()

    partition_name = nc.partition_id_tensor.name if nc.partition_id_tensor else None
    in_names, out_names, out_avals, zero_outs = [], [], [], []
    for alloc in nc.m.functions[0].allocations:
        if not isinstance(alloc, mybir.MemoryLocationSet):
            continue
        name = alloc.memorylocations[0].name
        if alloc.kind == "ExternalInput":
            if name != partition_name:
                in_names.append(name)
        elif alloc.kind == "ExternalOutput":
            out_names.append(name)
            shape = tuple(alloc.tensor_shape)
            dtype = mybir.dt.np(alloc.dtype)
            out_avals.append(jax.core.ShapedArray(shape, dtype))
            zero_outs.append(np.zeros(shape, dtype))
    n_params = len(in_names)
    n_outs = len(out_avals)
    in_names.extend(out_names)
    if partition_name is not None:
        in_names.append(partition_name)
    donate = tuple(range(n_params, n_params + n_outs)) if donate else ()

    def _body(*args):
        operands = list(args)
        if partition_name is not None:
            operands.append(bass2jax.partition_id_tensor())
        outs = bass2jax._bass_exec_p.bind(
            *operands,
            out_avals=tuple(out_avals),
            in_names=tuple(in_names),
            out_names=tuple(out_names),
            lowering_input_output_aliases=(),
            sim_require_finite=True,
            sim_require_nnan=True,
            nc=nc,
        )
        return tuple(outs)

    if devices is None:
        devices = jax.devices()[:n_cores]
    assert len(devices) == n_cores
    mesh = Mesh(np.asarray(devices), ("core",))
    in_specs = (PartitionSpec("core"),) * (n_params + n_outs)
    out_specs = (PartitionSpec("core"),) * len(out_names)
    sharded = jax.jit(
        shard_map(_body, mesh=mesh, in_specs=in_specs, out_specs=out_specs,
                  check_rep=False),
        donate_argnums=donate, keep_unused=True)

    shard = NamedSharding(mesh, PartitionSpec("core"))
    concat_in = [
        jax.device_put(
            np.concatenate([np.asarray(in_maps[c][name]) for c in range(n_cores)],
                           axis=0), shard)
        for name in in_names[:n_params]
    ]
    jax.block_until_ready(concat_in)

    def zeros():
        return [
            jax.device_put(np.zeros((n_cores * z.shape[0], *z.shape[1:]), z.dtype),
                           shard)
            for z in zero_outs
        ]

    out_arrs = jax.block_until_ready(sharded(*concat_in, *zeros()))
    exec_ns = None
    if time_iters > 0:
        m_lo, m_hi = 4, 4 + max(4, time_iters)

        def run_m(m):
            zs = [zeros() for _ in range(m)]
            jax.block_until_ready(zs)
            t0 = time.perf_counter()
            rs = [sharded(*concat_in, *z) for z in zs]
            jax.block_until_ready(rs)
            return time.perf_counter() - t0

        run_m(2)  # warm
        lo = min(run_m(m_lo) for _ in range(2))
        hi = min(run_m(m_hi) for _ in range(2))
        exec_ns = int(max(hi - lo, 0) / (m_hi - m_lo) * 1e9)
    results = [
        {name: np.asarray(out_arrs[i]).reshape(n_cores, *out_avals[i].shape)[c]
         for i, name in enumerate(out_names)}
        for c in range(n_cores)
    ]
    return results, exec_ns


def _kernel_impl(inputs, cfg, devices=None, donate=True, iters=0):
    in_maps, meta = preprocess(cfg=cfg, **inputs)
    nc = build(cfg, meta)
    results, exec_ns = _run_pjrt(nc, in_maps, NCORES, time_iters=iters,
                                 devices=devices, donate=donate)
    return _assemble_y(results, cfg), exec_ns


def kernel(x, edge_index, W_in, b_in, convs_W, W_out, b_out):
    global LAST_EXEC_NS
    iters = int(os.environ.get("KERNEL_TIME_ITERS", "0"))
    y, exec_ns = _kernel_impl(
        dict(x=x, edge_index=edge_index, W_in=W_in, b_in=b_in,
             convs_W=convs_W, W_out=W_out, b_out=b_out),
        _cfg_full(), iters=iters)
    LAST_EXEC_NS = exec_ns
    return y


# revision 9
# speedup vs baseline: 2.4909x; 1.1966x over previous
"""GCNII node regressor on 8 trn2 NeuronCores (Bass/Tile kernel), v2.

Strategy: nodes row-sharded across 8 cores (12500 each); edges partitioned
by dst core so the segment-sum is local; small weights replicated.

v2 replaces the ap_gather (GPSIMD, ~26ns/edge) message gather of v1 with
the stock indirect DMA (gpsimd indirect_dma_start -> dma_memcopy_indirect
ucode): h lives NODE-major ([node, 128 feat] fp16, 256B rows) in per-core
DRAM tables (AllGather'd per source quarter), and each slot chunk is
gathered straight into [slot%128 partition, slot//128, feat] SBUF layout
by the DMA engines -- also eliminating the per-group PE transposes and
the 100KB/partition SBUF window of v1.  The scatter side keeps v1's
host-prebuilt one-hot*weight "S" blocks, now fp16 and CW=128 dst columns
per cell: s = 0.9*Ahat@h + 0.1*h0 accumulates in PSUM per cell and lands
in an SBUF-resident s accumulator (no DRAM staging).  The layer update
h+ = relu(s @ W_eff) with W_eff = (1-b)I + b*W runs fp32 from s_sb; the
fp16 result is PE-transposed back to node-major and written to the next
h shard, with per-quarter AllGathers pipelined into the finish phase.

Everything irregular (degrees, normalization, edge sorting into uniform
per-(window, dst-cell) streams, gather index / S-block streams) is
precomputed on the host in numpy; the device program is straight-line and
identical on all 8 cores (SPMD).
"""

import math
import os

import numpy as np
import ml_dtypes

# ---------------- problem constants (full size, hardcoded) ----------------
N = 100000
E = 1600000
IN_DIM = 256
HID = 128
LAYERS = 8
ALPHA = 0.1
THETA = 0.5
NCORES = 8

P = 128          # partitions
NW = 4           # source windows per layer (quarters)
CW = 256         # aggregation cell dst width
YB = 512         # output block width
SB = 8           # S groups per stream tile

H_DT = np.float16         # h table / gather / S dtype

LAST_EXEC_NS = None


class Cfg:
    def __init__(self, n, e, in_dim, hid, layers):
        assert n % NCORES == 0
        self.n, self.e, self.in_dim, self.hid, self.layers = n, e, in_dim, hid, layers
        self.n_per = n // NCORES
        self.n_pad = ((self.n_per + P - 1) // P) * P
        # source quarters: q0..q2 of size qs (multiple of 128), q3 remainder
        qs = ((self.n_per + 3) // 4 + P - 1) // P * P
        self.qs = qs
        self.n_pad = ((self.n_per + P - 1) // P) * P
        q3 = self.n_pad - 3 * qs                # q3 padded so quarters tile n_pad
        assert 0 < q3 <= qs and q3 % P == 0, (self.n_per, qs, q3)
        self.qsizes = [qs, qs, qs, q3]          # rows per rank-quarter (q3 padded)
        for sz in self.qsizes:
            assert 8 * sz <= 32767, "table row index must fit int16"
        self.NB = self.n_pad // P               # dst 128-blocks per core
        self.NC2 = self.n_pad // CW             # dst cell blocks per core
        self.betas = [float(np.log(THETA / (i + 1) + 1.0)) for i in range(layers)]

    def wsize(self, w):
        """window w (= quarter) node count (table rows)"""
        return 8 * self.qsizes[w]


def _cfg_full():
    return Cfg(N, E, IN_DIM, HID, LAYERS)


# ---------------- host preprocessing ----------------

def preprocess(x, edge_index, W_in, b_in, convs_W, W_out, b_out, cfg):
    """Build per-core input maps + shared structure metadata."""
    n, n_per, qs = cfg.n, cfg.n_per, cfg.qs
    qsz = np.asarray(cfg.qsizes, np.int64)
    row = np.asarray(edge_index[0], np.int64)
    col = np.asarray(edge_index[1], np.int64)

    deg = np.bincount(col, minlength=n).astype(np.float32) + 1.0
    dinv = (1.0 / np.sqrt(deg)).astype(np.float32)
    wt = ((1.0 - ALPHA) * dinv[row] * dinv[col]).astype(np.float32)

    # append self loops as explicit edges
    allv = np.arange(n, dtype=np.int64)
    row_a = np.concatenate([row, allv])
    col_a = np.concatenate([col, allv])
    wt_a = np.concatenate([wt, ((1.0 - ALPHA) * dinv * dinv).astype(np.float32)])

    # source window (quarter) + table row within window
    r_s = row_a // n_per
    i_s = row_a % n_per
    q_s = np.minimum(i_s // qs, 3)
    w_s = q_s
    slot = (r_s * qsz[q_s] + (i_s - q_s * qs)).astype(np.int64)

    r_d = col_a // n_per
    dloc = col_a % n_per
    b_idx = dloc // CW                           # dst cell block
    dcol = (dloc % CW).astype(np.int64)

    NB = cfg.NC2
    counts = np.zeros((NCORES, NW, NB), np.int64)
    np.add.at(counts, (r_d, w_s, b_idx), 1)
    n_cb = counts.max(axis=0)                    # [NW, NB]
    n_cb = np.maximum(32 * ((n_cb + 31) // 32), 32)
    L = n_cb.sum(axis=1)
    n_cb[:, NB - 1] += (-L) % P                  # slot streams multiple of 128
    L = n_cb.sum(axis=1)                         # [NW]

    offs = np.zeros((NW, NB + 1), np.int64)
    offs[:, 1:] = np.cumsum(n_cb, axis=1)

    key = (r_d * NW + w_s) * NB + b_idx
    order = np.argsort(key, kind="stable")
    sk = key[order]
    grp_first = np.r_[0, np.flatnonzero(np.diff(sk)) + 1]
    grp_id = np.zeros(len(sk), np.int64)
    grp_id[grp_first[1:]] = 1
    grp_id = np.cumsum(grp_id)
    rank_in_cell = np.arange(len(sk)) - grp_first[grp_id]
    pos = offs[w_s[order], b_idx[order]] + rank_in_cell

    in_maps = []
    for r in range(NCORES):
        m = {}
        xs = np.zeros((cfg.n_pad, cfg.in_dim), np.float32)
        xs[:n_per] = np.asarray(x[r * n_per:(r + 1) * n_per], np.float32)
        m["x"] = xs
        sel_r = r_d[order] == r
        for c in range(NW):
            sel = sel_r & (w_s[order] == c)
            p = pos[sel]
            idx_arr = np.zeros((P, int(L[c]) // P), np.int32)
            idx_arr[p % P, p // P] = slot[order][sel].astype(np.int32)
            sarr = np.zeros((P, (L[c] // P) * CW), H_DT)
            sarr[p % P, (p // P) * CW + dcol[order][sel]] = wt_a[order][sel]
            m[f"idx{c}"] = idx_arr
            m[f"sblk{c}"] = sarr
        m["w_in"] = np.asarray(W_in, np.float32)
        m["b_in"] = np.asarray(b_in, np.float32).reshape(cfg.hid, 1)
        weff = np.concatenate(
            [((1.0 - cfg.betas[i]) * np.eye(cfg.hid, dtype=np.float32)
              + cfg.betas[i] * np.asarray(convs_W[i], np.float32))
             for i in range(cfg.layers)], axis=1)
        m["w_eff"] = weff
        m["w_out"] = np.asarray(W_out, np.float32).reshape(cfg.hid, 1)
        m["b_out"] = np.asarray(b_out, np.float32).reshape(1, 1)
        in_maps.append(m)

    return in_maps, {"n_cb": n_cb, "L": L}


# ---------------- device program ----------------

def build(cfg, meta, debug=False):
    import concourse.bass as bass
    import concourse.mybir as mybir
    from concourse import bacc
    from concourse.masks import make_identity
    from concourse.tile import TileContext
    from contextlib import ExitStack

    f32 = mybir.dt.float32
    f16 = mybir.dt.float16
    i32 = mybir.dt.int32
    Relu = mybir.ActivationFunctionType.Relu
    n_cb, L = meta["n_cb"], meta["L"]
    hid, in_dim = cfg.hid, cfg.in_dim
    qsz = cfg.qsizes

    nc = bacc.Bacc("TRN2", target_bir_lowering=False, debug=debug)

    x_in = nc.dram_tensor("x", [cfg.n_pad, in_dim], f32, kind="ExternalInput")
    idx_in, s_in = [], []
    for c in range(NW):
        idx_in.append(nc.dram_tensor(f"idx{c}", [P, int(L[c]) // P], i32,
                                     kind="ExternalInput"))
        s_in.append(nc.dram_tensor(f"sblk{c}", [P, (int(L[c]) // P) * CW], f16,
                                   kind="ExternalInput"))
    w_in_t = nc.dram_tensor("w_in", [in_dim, hid], f32, kind="ExternalInput")
    b_in_t = nc.dram_tensor("b_in", [hid, 1], f32, kind="ExternalInput")
    w_eff_t = nc.dram_tensor("w_eff", [hid, cfg.layers * hid], f32,
                             kind="ExternalInput")
    w_out_t = nc.dram_tensor("w_out", [hid, 1], f32, kind="ExternalInput")
    b_out_t = nc.dram_tensor("b_out", [1, 1], f32, kind="ExternalInput")
    y_out = nc.dram_tensor("y", [1, cfg.n_pad], f32, kind="ExternalOutput")

    rg = [list(range(NCORES))]

    with TileContext(nc) as tc, ExitStack() as ctx:
        const = ctx.enter_context(tc.tile_pool(name="const", bufs=1))
        gathp = ctx.enter_context(tc.tile_pool(name="gath", bufs=40))
        sblkp = ctx.enter_context(tc.tile_pool(name="sblk", bufs=4))
        idxp = ctx.enter_context(tc.tile_pool(name="idxt", bufs=2))
        xiop = ctx.enter_context(tc.tile_pool(name="xio", bufs=2))
        xtp = ctx.enter_context(tc.tile_pool(name="xt", bufs=3))
        wbp = ctx.enter_context(tc.tile_pool(name="wb", bufs=3))
        hbp = ctx.enter_context(tc.tile_pool(name="hb", bufs=2))
        h4p = ctx.enter_context(tc.tile_pool(name="h4", bufs=2))
        ytp = ctx.enter_context(tc.tile_pool(name="yt", bufs=2))
        pagg = ctx.enter_context(tc.tile_pool(name="pagg", bufs=3, space="PSUM"))
        ptr = ctx.enter_context(tc.tile_pool(name="ptr", bufs=3, space="PSUM"))
        pmisc = ctx.enter_context(tc.tile_pool(name="pmisc", bufs=2, space="PSUM"))
        dram = ctx.enter_context(tc.tile_pool(name="dram", bufs=1, space="DRAM"))

        # node-major per-quarter h shard + AG'd tables (fp16)
        h_shard = []
        h_table = [None] * 4
        for q in range(4):
            h_shard.append(dram.tile([qsz[q], P], f16, tag=f"h_shard{q}",
                                     name=f"h_shard{q}"))

        id_f = const.tile([P, P], f32, tag="id_f", name="id_f")
        make_identity(nc, id_f[:])
        w_in_sb = const.tile([P, (in_dim // P) * hid], f32, tag="w_in",
                             name="w_in_sb")
        for k in range(in_dim // P):
            nc.sync.dma_start(out=w_in_sb[:, k * hid:(k + 1) * hid],
                              in_=w_in_t[k * P:(k + 1) * P, :])
        b_in_sb = const.tile([P, 1], f32, tag="b_in", name="b_in_sb")
        nc.sync.dma_start(out=b_in_sb[:], in_=b_in_t[:])
        b_in_s = const.tile([P, 1], f32, tag="b_in_s", name="b_in_s")
        nc.vector.tensor_scalar_mul(b_in_s[:], b_in_sb[:], ALPHA)
        w_eff_sb = const.tile([P, cfg.layers * hid], f32, tag="w_eff",
                              name="w_eff_sb")
        nc.sync.dma_start(out=w_eff_sb[:], in_=w_eff_t[:])
        w_out_sb = const.tile([P, 1], f32, tag="w_out", name="w_out_sb")
        nc.sync.dma_start(out=w_out_sb[:], in_=w_out_t[:])
        b_out_sb = const.tile([1, 1], f32, tag="b_out", name="b_out_sb")
        nc.sync.dma_start(out=b_out_sb[:], in_=b_out_t[:])

        # SBUF-resident accumulators
        s_sb = const.tile([P, cfg.n_pad], f32, tag="s_sb", name="s_sb")
        h0a_sb = const.tile([P, cfg.n_pad], f32, tag="h0a", name="h0a_sb")

        def emit_ag(q):
            tab = dram.tile([NCORES * qsz[q], P], f16, tag=f"h_table{q}",
                            name=f"h_table{q}", addr_space="Shared", bufs=2)
            nc.gpsimd.collective_compute(
                "AllGather", mybir.AluOpType.bypass, replica_groups=rg,
                ins=[h_shard[q][:, :].opt()],
                outs=[tab[:].opt()])
            h_table[q] = tab

        def write_h_rows(hs4, nblk, lo):
            """DMA hs4[:, :nblk, :] (node%128, blk, feat) to node-major
            h_shard rows [lo, lo+128*nblk), splitting at quarter bounds."""
            done = 0
            while done < nblk:
                pos = lo + done * P
                q = min(pos // cfg.qs, 3)
                co = pos - q * cfg.qs
                take = min(nblk - done, (qsz[q] - co) // P)
                if take <= 0:
                    break                 # pad rows past the real nodes
                dst = h_shard[q][co:co + take * P, :].rearrange(
                    "(j p) d -> p j d", p=P)
                nc.sync.dma_start(out=dst, in_=hs4[:, done:done + take, :])
                done += take

        def fire_ags(lo, hi):
            for q in range(3):
                if lo < (q + 1) * cfg.qs <= hi:
                    emit_ag(q)
            if hi >= cfg.n_pad:
                emit_ag(3)

        # ---------------- init: h0 = relu(x@W_in + b_in) ----------------
        nblk_grp = 4
        for nt0 in range(0, cfg.NB, nblk_grp):
            nb = min(nblk_grp, cfg.NB - nt0)
            hs4 = h4p.tile([P, nblk_grp, P], f16, tag="hs4", name="hs4")
            for j in range(nb):
                nt = nt0 + j
                x_tile = xiop.tile([P, in_dim], f32, tag="x", name="x_tile")
                nc.sync.dma_start(out=x_tile[:], in_=x_in[nt * P:(nt + 1) * P, :])
                xts = []
                for k in range(in_dim // P):
                    xt_ps = ptr.tile([P, P], f32, tag="ptr", name="xt_ps")
                    nc.tensor.transpose(xt_ps[:], x_tile[:, k * P:(k + 1) * P],
                                        id_f[:])
                    xt_sb = xtp.tile([P, P], f32, tag="xt", name="xt_sb")
                    nc.vector.tensor_copy(out=xt_sb[:], in_=xt_ps[:])
                    xts.append(xt_sb)
                ph0 = pmisc.tile([P, YB], f32, tag="pmisc", name="pm")
                nk = in_dim // P
                for k in range(nk):
                    nc.tensor.matmul(ph0[:, :P],
                                     lhsT=w_in_sb[:, k * hid:(k + 1) * hid],
                                     rhs=xts[k][:], start=(k == 0),
                                     stop=(k == nk - 1))
                # alpha * h0 stays feat-major in SBUF
                nc.scalar.activation(h0a_sb[:, nt * P:(nt + 1) * P], ph0[:, :P],
                                     Relu, bias=b_in_s[:], scale=ALPHA)
                # h -> transpose to node-major, cast fp16 on copy-out
                hb = wbp.tile([P, P], f32, tag="wb", name="hbi")
                nc.scalar.activation(hb[:], ph0[:, :P], Relu, bias=b_in_sb[:])
                pt = ptr.tile([P, P], f32, tag="ptr", name="pt")
                nc.tensor.transpose(pt[:], hb[:], id_f[:])
                nc.vector.tensor_copy(out=hs4[:, j, :], in_=pt[:])
            write_h_rows(hs4, nb, nt0 * P)
            fire_ags(nt0 * P, (nt0 + nb) * P)

        # ---------------- layers ----------------
        for layer in range(cfg.layers):
            last = layer == cfg.layers - 1
            tables = list(h_table)

            def finish_block(b, tables=tables, layer=layer, last=last):
                """YB-wide output block b of s_sb is complete"""
                w = min(YB, cfg.n_pad - b * YB)
                cols = slice(b * YB, b * YB + w)
                ps = pmisc.tile([P, YB], f32, tag="pmisc", name="pm")
                nc.tensor.matmul(ps[:, :w],
                                 lhsT=w_eff_sb[:, layer * hid:(layer + 1) * hid],
                                 rhs=s_sb[:, cols], start=True, stop=True)
                if not last:
                    hb = hbp.tile([P, YB], f32, tag="hb", name="hb")
                    nc.scalar.activation(hb[:, :w], ps[:, :w], Relu)
                    hs4 = h4p.tile([P, YB // P, P], f16, tag="hs4f", name="hs4f")
                    for j in range(w // P):
                        pt = ptr.tile([P, P], f32, tag="ptr", name="pt")
                        nc.tensor.transpose(pt[:], hb[:, j * P:(j + 1) * P],
                                            id_f[:])
                        nc.vector.tensor_copy(out=hs4[:, j, :], in_=pt[:])
                    write_h_rows(hs4, w // P, b * YB)
                    fire_ags(b * YB, b * YB + w)
                else:
                    h8 = wbp.tile([P, YB], f32, tag="wb", name="wb")
                    nc.scalar.activation(h8[:, :w], ps[:, :w], Relu)
                    psy = pmisc.tile([P, YB], f32, tag="pmisc", name="pm")
                    nc.tensor.matmul(psy[0:1, :w], lhsT=w_out_sb[:, 0:1],
                                     rhs=h8[:, :w], start=True, stop=True)
                    yt = ytp.tile([1, YB], f32, tag="yt", name="yt")
                    nc.vector.tensor_tensor(
                        out=yt[0:1, :w], in0=psy[0:1, :w],
                        in1=b_out_sb[0:1, 0:1].to_broadcast([1, w]),
                        op=mybir.AluOpType.add)
                    nc.sync.dma_start(out=y_out[0:1, b * YB:b * YB + w],
                                      in_=yt[0:1, :w])

            for c in range(NW):
                Lc = int(L[c])
                ng = Lc // P
                it_win = idxp.tile([P, ng], i32, tag=f"itw{c}", name="it_win")
                nc.sync.dma_start(out=it_win[:], in_=idx_in[c][:])
                gt_tiles = {}
                s_tiles = [None] * ((ng + SB - 1) // SB)

                def ensure_group(g, c=c, it_win=it_win, gt_tiles=gt_tiles,
                                 tables=tables):
                    if g in gt_tiles:
                        return
                    gt = gathp.tile([P, P], f16, tag="gt", name="gt")
                    nc.gpsimd.indirect_dma_start(
                        out=gt[:], out_offset=None,
                        in_=tables[c][:],
                        in_offset=bass.IndirectOffsetOnAxis(
                            ap=it_win[:, g:g + 1], axis=0))
                    gt_tiles[g] = gt

                def ensure_s(sb, c=c, ng=ng, s_tiles=s_tiles):
                    if s_tiles[sb] is not None:
                        return
                    st = sblkp.tile([P, SB * CW], f16, tag="st", name="st")
                    lo = sb * SB * CW
                    ncols = min(SB * CW, ng * CW - lo)
                    nc.sync.dma_start(out=st[:, :ncols],
                                      in_=s_in[c][:, lo:lo + ncols])
                    s_tiles[sb] = st

                cur = 0
                for b in range(cfg.NC2):
                    ps_b = pagg.tile([P, CW], f32, tag="ps_b", name="ps_b")
                    n_slots = int(n_cb[c][b])
                    first = True
                    left = n_slots
                    while left > 0:
                        g, p0 = cur // P, cur % P
                        ln = 0
                        for sz in (128, 64, 32):
                            if p0 % sz == 0 and left >= sz and p0 + sz <= P:
                                ln = sz
                                break
                        assert ln, (p0, left)
                        ensure_group(g)
                        ensure_s(g // SB)
                        st = s_tiles[g // SB]
                        so = (g % SB) * CW
                        nc.tensor.matmul(
                            ps_b[:],
                            lhsT=gt_tiles[g][p0:p0 + ln, :],
                            rhs=st[p0:p0 + ln, so:so + CW],
                            start=first, stop=(ln == left),
                            tile_position=(p0, 0))
                        first = False
                        cur += ln
                        left -= ln
                    cols = slice(b * CW, (b + 1) * CW)
                    if c == 0:
                        nc.vector.tensor_add(out=s_sb[:, cols],
                                             in0=h0a_sb[:, cols], in1=ps_b[:])
                    else:
                        nc.vector.tensor_add(out=s_sb[:, cols],
                                             in0=s_sb[:, cols], in1=ps_b[:])
                    if c == NW - 1 and (((b + 1) * CW) % YB == 0 or
                                        b == cfg.NC2 - 1):
                        finish_block((b * CW) // YB)

    nc.compile()
    return nc


# ---------------- top level ----------------

def _assemble_y(results, cfg):
    parts = []
    for r in range(NCORES):
        y = np.asarray(results[r]["y"], np.float32).reshape(-1)
        parts.append(y[:cfg.n_per])
    return np.concatenate(parts)


def _run_pjrt(nc, in_maps, n_cores, time_iters=0, devices=None, donate=True):
    """Execute the bass program on the NeuronCores via PJRT (the axon
    redirect path of run_bass_kernel_spmd), with inputs pre-staged on
    device.  Mirrors concourse.bass2jax.run_bass_via_pjrt (multi-core).

    The axon dispatch floor is ~80ms/call, so single-call wall time says
    nothing about device time; with time_iters > 0 the marginal cost per
    execute between pipelined batches of M_lo and M_hi back-to-back
    calls is reported: device exec time plus ~1ms per-call dispatch (an
    honest upper bound on HW time).
    """
    import time
    import jax
    from jax.sharding import Mesh, NamedSharding, PartitionSpec
    from jax.experimental.shard_map import shard_map
    from concourse import bass2jax, mybir

    bass2jax.install_neuronx_cc_hook()

    partition_name = nc.partition_id_tensor.name if nc.partition_id_tensor else None
    in_names, out_names, out_avals, zero_outs = [], [], [], []
    for alloc in nc.m.functions[0].allocations:
        if not isinstance(alloc, mybir.MemoryLocationSet):
            continue
        name = alloc.memorylocations[0].name
        if alloc.kind == "ExternalInput":
            if name != partition_name:
                in_names.append(name)
        elif alloc.kind == "ExternalOutput":
            out_names.append(name)
            shape = tuple(alloc.tensor_shape)
            dtype = mybir.dt.np(alloc.dtype)
            out_avals.append(jax.core.ShapedArray(shape, dtype))
            zero_outs.append(np.zeros(shape, dtype))
    n_params = len(in_names)
    n_outs = len(out_avals)
    in_names.extend(out_names)
    if partition_name is not None:
        in_names.append(partition_name)
    donate = tuple(range(n_params, n_params + n_outs)) if donate else ()

    def _body(*args):
        operands = list(args)
        if partition_name is not None:
            operands.append(bass2jax.partition_id_tensor())
        outs = bass2jax._bass_exec_p.bind(
            *operands,
            out_avals=tuple(out_avals),
            in_names=tuple(in_names),
            out_names=tuple(out_names),
            lowering_input_output_aliases=(),
            sim_require_finite=True,
            sim_require_nnan=True,
            nc=nc,
        )
        return tuple(outs)

    if devices is None:
        devices = jax.devices()[:n_cores]
    assert len(devices) == n_cores
    mesh = Mesh(np.asarray(devices), ("core",))
    in_specs = (PartitionSpec("core"),) * (n_params + n_outs)
    out_specs = (PartitionSpec("core"),) * len(out_names)
    sharded = jax.jit(
        shard_map(_body, mesh=mesh, in_specs=in_specs, out_specs=out_specs,
                  check_rep=False),
        donate_argnums=donate, keep_unused=True)

    shard = NamedSharding(mesh, PartitionSpec("core"))
    concat_in = [
        jax.device_put(
            np.concatenate([np.asarray(in_maps[c][name]) for c in range(n_cores)],
                           axis=0), shard)
        for name in in_names[:n_params]
    ]
    jax.block_until_ready(concat_in)

    def zeros():
        return [
            jax.device_put(np.zeros((n_cores * z.shape[0], *z.shape[1:]), z.dtype),
                           shard)
            for z in zero_outs
        ]

    out_arrs = jax.block_until_ready(sharded(*concat_in, *zeros()))
    exec_ns = None
    if time_iters > 0:
        m_lo, m_hi = 4, 4 + max(4, time_iters)

        def run_m(m):
            zs = [zeros() for _ in range(m)]
            jax.block_until_ready(zs)
            t0 = time.perf_counter()
            rs = [sharded(*concat_in, *z) for z in zs]
            jax.block_until_ready(rs)
            return time.perf_counter() - t0

        run_m(2)  # warm
        lo = min(run_m(m_lo) for _ in range(2))
        hi = min(run_m(m_hi) for _ in range(2))
        exec_ns = int(max(hi - lo, 0) / (m_hi - m_lo) * 1e9)
    results = [
        {name: np.asarray(out_arrs[i]).reshape(n_cores, *out_avals[i].shape)[c]
         for i, name in enumerate(out_names)}
        for c in range(n_cores)
    ]
    return results, exec_ns


def _kernel_impl(inputs, cfg, devices=None, donate=True, iters=0):
    in_maps, meta = preprocess(cfg=cfg, **inputs)
    nc = build(cfg, meta)
    results, exec_ns = _run_pjrt(nc, in_maps, NCORES, time_iters=iters,
                                 devices=devices, donate=donate)
    return _assemble_y(results, cfg), exec_ns


def kernel(x, edge_index, W_in, b_in, convs_W, W_out, b_out):
    global LAST_EXEC_NS
    iters = int(os.environ.get("KERNEL_TIME_ITERS", "0"))
    y, exec_ns = _kernel_impl(
        dict(x=x, edge_index=edge_index, W_in=W_in, b_in=b_in,
             convs_W=convs_W, W_out=W_out, b_out=b_out),
        _cfg_full(), iters=iters)
    LAST_EXEC_NS = exec_ns
    return y


# revision 16
# speedup vs baseline: 2.5211x; 1.0121x over previous
"""GCNII node regressor on 8 trn2 NeuronCores (Bass/Tile kernel), v2.

Strategy: nodes row-sharded across 8 cores (12500 each); edges partitioned
by dst core so the segment-sum is local; small weights replicated.

v2 replaces the ap_gather (GPSIMD, ~26ns/edge) message gather of v1 with
the stock indirect DMA (gpsimd indirect_dma_start -> dma_memcopy_indirect
ucode): h lives NODE-major ([node, 128 feat] fp16, 256B rows) in per-core
DRAM tables (AllGather'd per source quarter), and each slot chunk is
gathered straight into [slot%128 partition, slot//128, feat] SBUF layout
by the DMA engines -- also eliminating the per-group PE transposes and
the 100KB/partition SBUF window of v1.  The scatter side keeps v1's
host-prebuilt one-hot*weight "S" blocks, now fp16 and CW=128 dst columns
per cell: s = 0.9*Ahat@h + 0.1*h0 accumulates in PSUM per cell and lands
in an SBUF-resident s accumulator (no DRAM staging).  The layer update
h+ = relu(s @ W_eff) with W_eff = (1-b)I + b*W runs fp32 from s_sb; the
fp16 result is PE-transposed back to node-major and written to the next
h shard, with per-quarter AllGathers pipelined into the finish phase.

Everything irregular (degrees, normalization, edge sorting into uniform
per-(window, dst-cell) streams, gather index / S-block streams) is
precomputed on the host in numpy; the device program is straight-line and
identical on all 8 cores (SPMD).
"""

import math
import os

import numpy as np
import ml_dtypes

# ---------------- problem constants (full size, hardcoded) ----------------
N = 100000
E = 1600000
IN_DIM = 256
HID = 128
LAYERS = 8
ALPHA = 0.1
THETA = 0.5
NCORES = 8

P = 128          # partitions
NW = 4           # source windows per layer (quarters)
CW = 256         # aggregation cell dst width
YB = 512         # output block width
SB = 16          # S groups per stream tile

H_DT = np.float16         # h table / gather / S dtype

LAST_EXEC_NS = None


class Cfg:
    def __init__(self, n, e, in_dim, hid, layers):
        assert n % NCORES == 0
        self.n, self.e, self.in_dim, self.hid, self.layers = n, e, in_dim, hid, layers
        self.n_per = n // NCORES
        self.n_pad = ((self.n_per + P - 1) // P) * P
        # source quarters: q0..q2 of size qs (multiple of 128), q3 remainder
        qs = ((self.n_per + 3) // 4 + P - 1) // P * P
        self.qs = qs
        self.n_pad = ((self.n_per + P - 1) // P) * P
        q3 = self.n_pad - 3 * qs                # q3 padded so quarters tile n_pad
        assert 0 < q3 <= qs and q3 % P == 0, (self.n_per, qs, q3)
        self.qsizes = [qs, qs, qs, q3]          # rows per rank-quarter (q3 padded)
        for sz in self.qsizes:
            assert 8 * sz <= 32767, "table row index must fit int16"
        self.NB = self.n_pad // P               # dst 128-blocks per core
        self.NC2 = self.n_pad // CW             # dst cell blocks per core
        self.betas = [float(np.log(THETA / (i + 1) + 1.0)) for i in range(layers)]

    def wsize(self, w):
        """window w (= quarter) node count (table rows)"""
        return 8 * self.qsizes[w]


def _cfg_full():
    return Cfg(N, E, IN_DIM, HID, LAYERS)


# ---------------- host preprocessing ----------------

def preprocess(x, edge_index, W_in, b_in, convs_W, W_out, b_out, cfg):
    """Build per-core input maps + shared structure metadata."""
    n, n_per, qs = cfg.n, cfg.n_per, cfg.qs
    qsz = np.asarray(cfg.qsizes, np.int64)
    row = np.asarray(edge_index[0], np.int64)
    col = np.asarray(edge_index[1], np.int64)

    deg = np.bincount(col, minlength=n).astype(np.float32) + 1.0
    dinv = (1.0 / np.sqrt(deg)).astype(np.float32)
    wt = ((1.0 - ALPHA) * dinv[row] * dinv[col]).astype(np.float32)

    # append self loops as explicit edges
    allv = np.arange(n, dtype=np.int64)
    row_a = np.concatenate([row, allv])
    col_a = np.concatenate([col, allv])
    wt_a = np.concatenate([wt, ((1.0 - ALPHA) * dinv * dinv).astype(np.float32)])

    # source window (quarter) + table row within window
    r_s = row_a // n_per
    i_s = row_a % n_per
    q_s = np.minimum(i_s // qs, 3)
    w_s = q_s
    slot = (r_s * qsz[q_s] + (i_s - q_s * qs)).astype(np.int64)

    r_d = col_a // n_per
    dloc = col_a % n_per
    b_idx = dloc // CW                           # dst cell block
    dcol = (dloc % CW).astype(np.int64)

    NB = cfg.NC2
    counts = np.zeros((NCORES, NW, NB), np.int64)
    np.add.at(counts, (r_d, w_s, b_idx), 1)
    n_cb = counts.max(axis=0)                    # [NW, NB]
    n_cb = np.maximum(32 * ((n_cb + 31) // 32), 32)
    L = n_cb.sum(axis=1)
    n_cb[:, NB - 1] += (-L) % P                  # slot streams multiple of 128
    L = n_cb.sum(axis=1)                         # [NW]

    offs = np.zeros((NW, NB + 1), np.int64)
    offs[:, 1:] = np.cumsum(n_cb, axis=1)

    key = (r_d * NW + w_s) * NB + b_idx
    order = np.argsort(key, kind="stable")
    sk = key[order]
    grp_first = np.r_[0, np.flatnonzero(np.diff(sk)) + 1]
    grp_id = np.zeros(len(sk), np.int64)
    grp_id[grp_first[1:]] = 1
    grp_id = np.cumsum(grp_id)
    rank_in_cell = np.arange(len(sk)) - grp_first[grp_id]
    pos = offs[w_s[order], b_idx[order]] + rank_in_cell

    in_maps = []
    for r in range(NCORES):
        m = {}
        xs = np.zeros((cfg.n_pad, cfg.in_dim), np.float32)
        xs[:n_per] = np.asarray(x[r * n_per:(r + 1) * n_per], np.float32)
        m["x"] = xs
        sel_r = r_d[order] == r
        for c in range(NW):
            sel = sel_r & (w_s[order] == c)
            p = pos[sel]
            idx_arr = np.zeros((P, int(L[c]) // P), np.int32)
            idx_arr[p % P, p // P] = slot[order][sel].astype(np.int32)
            sarr = np.zeros((P, (L[c] // P) * CW), H_DT)
            sarr[p % P, (p // P) * CW + dcol[order][sel]] = wt_a[order][sel]
            m[f"idx{c}"] = idx_arr
            m[f"sblk{c}"] = sarr
        m["w_in"] = np.asarray(W_in, np.float32)
        m["b_in"] = np.asarray(b_in, np.float32).reshape(cfg.hid, 1)
        weff = np.concatenate(
            [((1.0 - cfg.betas[i]) * np.eye(cfg.hid, dtype=np.float32)
              + cfg.betas[i] * np.asarray(convs_W[i], np.float32))
             for i in range(cfg.layers)], axis=1)
        m["w_eff"] = weff
        m["w_out"] = np.asarray(W_out, np.float32).reshape(cfg.hid, 1)
        m["b_out"] = np.asarray(b_out, np.float32).reshape(1, 1)
        in_maps.append(m)

    return in_maps, {"n_cb": n_cb, "L": L}


# ---------------- device program ----------------

def build(cfg, meta, debug=False):
    import concourse.bass as bass
    import concourse.mybir as mybir
    from concourse import bacc
    from concourse.masks import make_identity
    from concourse.tile import TileContext
    from contextlib import ExitStack

    f32 = mybir.dt.float32
    f16 = mybir.dt.float16
    i32 = mybir.dt.int32
    Relu = mybir.ActivationFunctionType.Relu
    n_cb, L = meta["n_cb"], meta["L"]
    hid, in_dim = cfg.hid, cfg.in_dim
    qsz = cfg.qsizes

    nc = bacc.Bacc("TRN2", target_bir_lowering=False, debug=debug)

    x_in = nc.dram_tensor("x", [cfg.n_pad, in_dim], f32, kind="ExternalInput")
    idx_in, s_in = [], []
    for c in range(NW):
        idx_in.append(nc.dram_tensor(f"idx{c}", [P, int(L[c]) // P], i32,
                                     kind="ExternalInput"))
        s_in.append(nc.dram_tensor(f"sblk{c}", [P, (int(L[c]) // P) * CW], f16,
                                   kind="ExternalInput"))
    w_in_t = nc.dram_tensor("w_in", [in_dim, hid], f32, kind="ExternalInput")
    b_in_t = nc.dram_tensor("b_in", [hid, 1], f32, kind="ExternalInput")
    w_eff_t = nc.dram_tensor("w_eff", [hid, cfg.layers * hid], f32,
                             kind="ExternalInput")
    w_out_t = nc.dram_tensor("w_out", [hid, 1], f32, kind="ExternalInput")
    b_out_t = nc.dram_tensor("b_out", [1, 1], f32, kind="ExternalInput")
    y_out = nc.dram_tensor("y", [1, cfg.n_pad], f32, kind="ExternalOutput")

    rg = [list(range(NCORES))]

    with TileContext(nc) as tc, ExitStack() as ctx:
        const = ctx.enter_context(tc.tile_pool(name="const", bufs=1))
        gathp = ctx.enter_context(tc.tile_pool(name="gath", bufs=56))
        sblkp = ctx.enter_context(tc.tile_pool(name="sblk", bufs=3))
        idxp = ctx.enter_context(tc.tile_pool(name="idxt", bufs=2))
        xiop = ctx.enter_context(tc.tile_pool(name="xio", bufs=2))
        xtp = ctx.enter_context(tc.tile_pool(name="xt", bufs=3))
        wbp = ctx.enter_context(tc.tile_pool(name="wb", bufs=3))
        hbp = ctx.enter_context(tc.tile_pool(name="hb", bufs=2))
        h4p = ctx.enter_context(tc.tile_pool(name="h4", bufs=2))
        ytp = ctx.enter_context(tc.tile_pool(name="yt", bufs=2))
        pagg = ctx.enter_context(tc.tile_pool(name="pagg", bufs=3, space="PSUM"))
        ptr = ctx.enter_context(tc.tile_pool(name="ptr", bufs=3, space="PSUM"))
        pmisc = ctx.enter_context(tc.tile_pool(name="pmisc", bufs=2, space="PSUM"))
        dram = ctx.enter_context(tc.tile_pool(name="dram", bufs=1, space="DRAM"))

        # node-major per-quarter h shard + AG'd tables (fp16)
        h_shard = []
        h_table = [None] * 4
        for q in range(4):
            h_shard.append(dram.tile([qsz[q], P], f16, tag=f"h_shard{q}",
                                     name=f"h_shard{q}"))

        id_f = const.tile([P, P], f32, tag="id_f", name="id_f")
        make_identity(nc, id_f[:])
        w_in_sb = const.tile([P, (in_dim // P) * hid], f32, tag="w_in",
                             name="w_in_sb")
        for k in range(in_dim // P):
            nc.sync.dma_start(out=w_in_sb[:, k * hid:(k + 1) * hid],
                              in_=w_in_t[k * P:(k + 1) * P, :])
        b_in_sb = const.tile([P, 1], f32, tag="b_in", name="b_in_sb")
        nc.sync.dma_start(out=b_in_sb[:], in_=b_in_t[:])
        b_in_s = const.tile([P, 1], f32, tag="b_in_s", name="b_in_s")
        nc.vector.tensor_scalar_mul(b_in_s[:], b_in_sb[:], ALPHA)
        w_eff_sb = const.tile([P, cfg.layers * hid], f32, tag="w_eff",
                              name="w_eff_sb")
        nc.sync.dma_start(out=w_eff_sb[:], in_=w_eff_t[:])
        w_out_sb = const.tile([P, 1], f32, tag="w_out", name="w_out_sb")
        nc.sync.dma_start(out=w_out_sb[:], in_=w_out_t[:])
        b_out_sb = const.tile([1, 1], f32, tag="b_out", name="b_out_sb")
        nc.sync.dma_start(out=b_out_sb[:], in_=b_out_t[:])

        # SBUF-resident accumulators
        s_sb = const.tile([P, cfg.n_pad], f32, tag="s_sb", name="s_sb")
        h0a_sb = const.tile([P, cfg.n_pad], f32, tag="h0a", name="h0a_sb")

        def emit_ag(q):
            tab = dram.tile([NCORES * qsz[q], P], f16, tag=f"h_table{q}",
                            name=f"h_table{q}", addr_space="Shared", bufs=2)
            nc.gpsimd.collective_compute(
                "AllGather", mybir.AluOpType.bypass, replica_groups=rg,
                ins=[h_shard[q][:, :].opt()],
                outs=[tab[:].opt()])
            h_table[q] = tab

        def write_h_rows(hs4, nblk, lo):
            """DMA hs4[:, :nblk, :] (node%128, blk, feat) to node-major
            h_shard rows [lo, lo+128*nblk), splitting at quarter bounds."""
            done = 0
            while done < nblk:
                pos = lo + done * P
                q = min(pos // cfg.qs, 3)
                co = pos - q * cfg.qs
                take = min(nblk - done, (qsz[q] - co) // P)
                if take <= 0:
                    break                 # pad rows past the real nodes
                dst = h_shard[q][co:co + take * P, :].rearrange(
                    "(j p) d -> p j d", p=P)
                nc.sync.dma_start(out=dst, in_=hs4[:, done:done + take, :])
                done += take

        def fire_ags(lo, hi):
            for q in range(3):
                if lo < (q + 1) * cfg.qs <= hi:
                    emit_ag(q)
            if hi >= cfg.n_pad:
                emit_ag(3)

        # ---------------- init: h0 = relu(x@W_in + b_in) ----------------
        nblk_grp = 4
        for nt0 in range(0, cfg.NB, nblk_grp):
            nb = min(nblk_grp, cfg.NB - nt0)
            hs4 = h4p.tile([P, nblk_grp, P], f16, tag="hs4", name="hs4")
            x4 = xiop.tile([P, nblk_grp, in_dim], f32, tag="x", name="x4")
            nc.sync.dma_start(
                out=x4[:, :nb, :],
                in_=x_in[nt0 * P:(nt0 + nb) * P, :].rearrange(
                    "(j p) d -> p j d", p=P))
            for j in range(nb):
                nt = nt0 + j
                x_tile = x4[:, j, :]
                xts = []
                for k in range(in_dim // P):
                    xt_ps = ptr.tile([P, P], f32, tag="ptr", name="xt_ps")
                    nc.tensor.transpose(xt_ps[:], x_tile[:, k * P:(k + 1) * P],
                                        id_f[:])
                    xt_sb = xtp.tile([P, P], f32, tag="xt", name="xt_sb")
                    nc.vector.tensor_copy(out=xt_sb[:], in_=xt_ps[:])
                    xts.append(xt_sb)
                ph0 = pmisc.tile([P, YB], f32, tag="pmisc", name="pm")
                nk = in_dim // P
                for k in range(nk):
                    nc.tensor.matmul(ph0[:, :P],
                                     lhsT=w_in_sb[:, k * hid:(k + 1) * hid],
                                     rhs=xts[k][:], start=(k == 0),
                                     stop=(k == nk - 1))
                # alpha * h0 stays feat-major in SBUF
                nc.scalar.activation(h0a_sb[:, nt * P:(nt + 1) * P], ph0[:, :P],
                                     Relu, bias=b_in_s[:], scale=ALPHA)
                # h -> transpose to node-major, cast fp16 on copy-out
                hb = wbp.tile([P, P], f32, tag="wb", name="hbi")
                nc.scalar.activation(hb[:], ph0[:, :P], Relu, bias=b_in_sb[:])
                pt = ptr.tile([P, P], f32, tag="ptr", name="pt")
                nc.tensor.transpose(pt[:], hb[:], id_f[:])
                nc.vector.tensor_copy(out=hs4[:, j, :], in_=pt[:])
            write_h_rows(hs4, nb, nt0 * P)
            fire_ags(nt0 * P, (nt0 + nb) * P)

        # ---------------- layers ----------------
        for layer in range(cfg.layers):
            last = layer == cfg.layers - 1
            tables = list(h_table)
            pending_ags = []      # (quarter, window3-cell to fire after)

            def finish_block(b, tables=tables, layer=layer, last=last,
                             pending_ags=pending_ags):
                """YB-wide output block b of s_sb is complete"""
                w = min(YB, cfg.n_pad - b * YB)
                cols = slice(b * YB, b * YB + w)
                ps = pmisc.tile([P, YB], f32, tag="pmisc", name="pm")
                nc.tensor.matmul(ps[:, :w],
                                 lhsT=w_eff_sb[:, layer * hid:(layer + 1) * hid],
                                 rhs=s_sb[:, cols], start=True, stop=True)
                if not last:
                    hb = hbp.tile([P, YB], f32, tag="hb", name="hb")
                    nc.scalar.activation(hb[:, :w], ps[:, :w], Relu)
                    hs4 = h4p.tile([P, YB // P, P], f16, tag="hs4f", name="hs4f")
                    for j in range(w // P):
                        pt = ptr.tile([P, P], f32, tag="ptr", name="pt")
                        nc.tensor.transpose(pt[:], hb[:, j * P:(j + 1) * P],
                                            id_f[:])
                        nc.vector.tensor_copy(out=hs4[:, j, :], in_=pt[:])
                    write_h_rows(hs4, w // P, b * YB)
                    # defer AG issuance a few cells so its SEQ wait (on the
                    # h_shard write chain) resolves before it can stall the
                    # Pool queue's gather stream
                    lo, hi = b * YB, b * YB + w
                    cur_cell = (hi - 1) // CW
                    for q in range(3):
                        if lo < (q + 1) * cfg.qs <= hi:
                            pending_ags.append((q, cur_cell + 6))
                    if hi >= cfg.n_pad:
                        pending_ags.append((3, cur_cell + 6))
                else:
                    h8 = wbp.tile([P, YB], f32, tag="wb", name="wb")
                    nc.scalar.activation(h8[:, :w], ps[:, :w], Relu)
                    psy = pmisc.tile([P, YB], f32, tag="pmisc", name="pm")
                    nc.tensor.matmul(psy[0:1, :w], lhsT=w_out_sb[:, 0:1],
                                     rhs=h8[:, :w], start=True, stop=True)
                    yt = ytp.tile([1, YB], f32, tag="yt", name="yt")
                    nc.vector.tensor_tensor(
                        out=yt[0:1, :w], in0=psy[0:1, :w],
                        in1=b_out_sb[0:1, 0:1].to_broadcast([1, w]),
                        op=mybir.AluOpType.add)
                    nc.sync.dma_start(out=y_out[0:1, b * YB:b * YB + w],
                                      in_=yt[0:1, :w])

            # prefetch every window's gather-index tile up front
            it_wins = []
            for c in range(NW):
                it_win = idxp.tile([P, int(L[c]) // P], i32, tag=f"itw{c}",
                                   name="it_win")
                nc.sync.dma_start(out=it_win[:], in_=idx_in[c][:])
                it_wins.append(it_win)

            for c in range(NW):
                Lc = int(L[c])
                ng = Lc // P
                it_win = it_wins[c]
                gt_tiles = {}
                s_tiles = [None] * ((ng + SB - 1) // SB)

                def ensure_group(g, c=c, it_win=it_win, gt_tiles=gt_tiles,
                                 tables=tables):
                    if g in gt_tiles:
                        return
                    gt = gathp.tile([P, P], f16, tag="gt", name="gt")
                    nc.gpsimd.indirect_dma_start(
                        out=gt[:], out_offset=None,
                        in_=tables[c][:],
                        in_offset=bass.IndirectOffsetOnAxis(
                            ap=it_win[:, g:g + 1], axis=0))
                    gt_tiles[g] = gt

                def ensure_s(sb, c=c, ng=ng, s_tiles=s_tiles):
                    if s_tiles[sb] is not None:
                        return
                    st = sblkp.tile([P, SB * CW], f16, tag="st", name="st")
                    lo = sb * SB * CW
                    ncols = min(SB * CW, ng * CW - lo)
                    nc.sync.dma_start(out=st[:, :ncols],
                                      in_=s_in[c][:, lo:lo + ncols])
                    s_tiles[sb] = st

                cur = 0
                for b in range(cfg.NC2):
                    ps_b = pagg.tile([P, CW], f32, tag="ps_b", name="ps_b")
                    n_slots = int(n_cb[c][b])
                    first = True
                    left = n_slots
                    while left > 0:
                        g, p0 = cur // P, cur % P
                        ln = 0
                        for sz in (128, 64, 32):
                            if p0 % sz == 0 and left >= sz and p0 + sz <= P:
                                ln = sz
                                break
                        assert ln, (p0, left)
                        ensure_group(g)
                        ensure_s(g // SB)
                        st = s_tiles[g // SB]
                        so = (g % SB) * CW
                        nc.tensor.matmul(
                            ps_b[:],
                            lhsT=gt_tiles[g][p0:p0 + ln, :],
                            rhs=st[p0:p0 + ln, so:so + CW],
                            start=first, stop=(ln == left),
                            tile_position=(p0, 0))
                        first = False
                        cur += ln
                        left -= ln
                    cols = slice(b * CW, (b + 1) * CW)
                    if c == 0:
                        nc.vector.tensor_add(out=s_sb[:, cols],
                                             in0=h0a_sb[:, cols], in1=ps_b[:])
                    else:
                        nc.vector.tensor_add(out=s_sb[:, cols],
                                             in0=s_sb[:, cols], in1=ps_b[:])
                    if c == NW - 1:
                        while pending_ags and pending_ags[0][1] <= b:
                            emit_ag(pending_ags.pop(0)[0])
                        if (((b + 1) * CW) % YB == 0 or b == cfg.NC2 - 1):
                            finish_block((b * CW) // YB)
            while pending_ags:
                emit_ag(pending_ags.pop(0)[0])

    nc.compile()
    return nc


# ---------------- top level ----------------

def _assemble_y(results, cfg):
    parts = []
    for r in range(NCORES):
        y = np.asarray(results[r]["y"], np.float32).reshape(-1)
        parts.append(y[:cfg.n_per])
    return np.concatenate(parts)


def _run_pjrt(nc, in_maps, n_cores, time_iters=0, devices=None, donate=True):
    """Execute the bass program on the NeuronCores via PJRT (the axon
    redirect path of run_bass_kernel_spmd), with inputs pre-staged on
    device.  Mirrors concourse.bass2jax.run_bass_via_pjrt (multi-core).

    The axon dispatch floor is ~80ms/call, so single-call wall time says
    nothing about device time; with time_iters > 0 the marginal cost per
    execute between pipelined batches of M_lo and M_hi back-to-back
    calls is reported: device exec time plus ~1ms per-call dispatch (an
    honest upper bound on HW time).
    """
    import time
    import jax
    from jax.sharding import Mesh, NamedSharding, PartitionSpec
    from jax.experimental.shard_map import shard_map
    from concourse import bass2jax, mybir

    bass2jax.install_neuronx_cc_hook()

    partition_name = nc.partition_id_tensor.name if nc.partition_id_tensor else None
    in_names, out_names, out_avals, zero_outs = [], [], [], []
    for alloc in nc.m.functions[0].allocations:
        if not isinstance(alloc, mybir.MemoryLocationSet):
            continue
        name = alloc.memorylocations[0].name
        if alloc.kind == "ExternalInput":
            if name != partition_name:
                in_names.append(name)
        elif alloc.kind == "ExternalOutput":
            out_names.append(name)
            shape = tuple(alloc.tensor_shape)
            dtype = mybir.dt.np(alloc.dtype)
            out_avals.append(jax.core.ShapedArray(shape, dtype))
            zero_outs.append(np.zeros(shape, dtype))
    n_params = len(in_names)
    n_outs = len(out_avals)
    in_names.extend(out_names)
    if partition_name is not None:
        in_names.append(partition_name)
    donate = tuple(range(n_params, n_params + n_outs)) if donate else ()

    def _body(*args):
        operands = list(args)
        if partition_name is not None:
            operands.append(bass2jax.partition_id_tensor())
        outs = bass2jax._bass_exec_p.bind(
            *operands,
            out_avals=tuple(out_avals),
            in_names=tuple(in_names),
            out_names=tuple(out_names),
            lowering_input_output_aliases=(),
            sim_require_finite=True,
            sim_require_nnan=True,
            nc=nc,
        )
        return tuple(outs)

    if devices is None:
        devices = jax.devices()[:n_cores]
    assert len(devices) == n_cores
    mesh = Mesh(np.asarray(devices), ("core",))
    in_specs = (PartitionSpec("core"),) * (n_params + n_outs)
    out_specs = (PartitionSpec("core"),) * len(out_names)
    sharded = jax.jit(
        shard_map(_body, mesh=mesh, in_specs=in_specs, out_specs=out_specs,
                  check_rep=False),
        donate_argnums=donate, keep_unused=True)

    shard = NamedSharding(mesh, PartitionSpec("core"))
    concat_in = [
        jax.device_put(
            np.concatenate([np.asarray(in_maps[c][name]) for c in range(n_cores)],
                           axis=0), shard)
        for name in in_names[:n_params]
    ]
    jax.block_until_ready(concat_in)

    def zeros():
        return [
            jax.device_put(np.zeros((n_cores * z.shape[0], *z.shape[1:]), z.dtype),
                           shard)
            for z in zero_outs
        ]

    out_arrs = jax.block_until_ready(sharded(*concat_in, *zeros()))
    exec_ns = None
    if time_iters > 0:
        m_lo, m_hi = 4, 4 + max(4, time_iters)

        def run_m(m):
            zs = [zeros() for _ in range(m)]
            jax.block_until_ready(zs)
            t0 = time.perf_counter()
            rs = [sharded(*concat_in, *z) for z in zs]
            jax.block_until_ready(rs)
            return time.perf_counter() - t0

        run_m(2)  # warm
        lo = min(run_m(m_lo) for _ in range(2))
        hi = min(run_m(m_hi) for _ in range(2))
        exec_ns = int(max(hi - lo, 0) / (m_hi - m_lo) * 1e9)
    results = [
        {name: np.asarray(out_arrs[i]).reshape(n_cores, *out_avals[i].shape)[c]
         for i, name in enumerate(out_names)}
        for c in range(n_cores)
    ]
    return results, exec_ns


def _kernel_impl(inputs, cfg, devices=None, donate=True, iters=0):
    in_maps, meta = preprocess(cfg=cfg, **inputs)
    nc = build(cfg, meta)
    results, exec_ns = _run_pjrt(nc, in_maps, NCORES, time_iters=iters,
                                 devices=devices, donate=donate)
    return _assemble_y(results, cfg), exec_ns


def kernel(x, edge_index, W_in, b_in, convs_W, W_out, b_out):
    global LAST_EXEC_NS
    iters = int(os.environ.get("KERNEL_TIME_ITERS", "0"))
    y, exec_ns = _kernel_impl(
        dict(x=x, edge_index=edge_index, W_in=W_in, b_in=b_in,
             convs_W=convs_W, W_out=W_out, b_out=b_out),
        _cfg_full(), iters=iters)
    LAST_EXEC_NS = exec_ns
    return y


# revision 17
# speedup vs baseline: 2.5531x; 1.0127x over previous
"""GCNII node regressor on 8 trn2 NeuronCores (Bass/Tile kernel), v2.

Strategy: nodes row-sharded across 8 cores (12500 each); edges partitioned
by dst core so the segment-sum is local; small weights replicated.

v2 replaces the ap_gather (GPSIMD, ~26ns/edge) message gather of v1 with
the stock indirect DMA (gpsimd indirect_dma_start -> dma_memcopy_indirect
ucode): h lives NODE-major ([node, 128 feat] fp16, 256B rows) in per-core
DRAM tables (AllGather'd per source quarter), and each slot chunk is
gathered straight into [slot%128 partition, slot//128, feat] SBUF layout
by the DMA engines -- also eliminating the per-group PE transposes and
the 100KB/partition SBUF window of v1.  The scatter side keeps v1's
host-prebuilt one-hot*weight "S" blocks, now fp16 and CW=128 dst columns
per cell: s = 0.9*Ahat@h + 0.1*h0 accumulates in PSUM per cell and lands
in an SBUF-resident s accumulator (no DRAM staging).  The layer update
h+ = relu(s @ W_eff) with W_eff = (1-b)I + b*W runs fp32 from s_sb; the
fp16 result is PE-transposed back to node-major and written to the next
h shard, with per-quarter AllGathers pipelined into the finish phase.

Everything irregular (degrees, normalization, edge sorting into uniform
per-(window, dst-cell) streams, gather index / S-block streams) is
precomputed on the host in numpy; the device program is straight-line and
identical on all 8 cores (SPMD).
"""

import os

import numpy as np

# ---------------- problem constants (full size, hardcoded) ----------------
N = 100000
E = 1600000
IN_DIM = 256
HID = 128
LAYERS = 8
ALPHA = 0.1
THETA = 0.5
NCORES = 8

P = 128          # partitions
NW = 4           # source windows per layer (quarters)
CW = 256         # aggregation cell dst width
YB = 512         # output block width
SB = 16          # S groups per stream tile

H_DT = np.float16         # h table / gather / S dtype

LAST_EXEC_NS = None


class Cfg:
    def __init__(self, n, e, in_dim, hid, layers):
        assert n % NCORES == 0
        self.n, self.e, self.in_dim, self.hid, self.layers = n, e, in_dim, hid, layers
        self.n_per = n // NCORES
        self.n_pad = ((self.n_per + P - 1) // P) * P
        # source quarters: q0..q2 of size qs (multiple of 128), q3 remainder
        qs = ((self.n_per + 3) // 4 + P - 1) // P * P
        self.qs = qs
        self.n_pad = ((self.n_per + P - 1) // P) * P
        q3 = self.n_pad - 3 * qs                # q3 padded so quarters tile n_pad
        assert 0 < q3 <= qs and q3 % P == 0, (self.n_per, qs, q3)
        self.qsizes = [qs, qs, qs, q3]          # rows per rank-quarter (q3 padded)
        for sz in self.qsizes:
            assert 8 * sz <= 32767, "table row index must fit int16"
        self.NB = self.n_pad // P               # dst 128-blocks per core
        self.NC2 = self.n_pad // CW             # dst cell blocks per core
        self.betas = [float(np.log(THETA / (i + 1) + 1.0)) for i in range(layers)]

    def wsize(self, w):
        """window w (= quarter) node count (table rows)"""
        return 8 * self.qsizes[w]


def _cfg_full():
    return Cfg(N, E, IN_DIM, HID, LAYERS)


# ---------------- host preprocessing ----------------

def preprocess(x, edge_index, W_in, b_in, convs_W, W_out, b_out, cfg):
    """Build per-core input maps + shared structure metadata."""
    n, n_per, qs = cfg.n, cfg.n_per, cfg.qs
    qsz = np.asarray(cfg.qsizes, np.int64)
    row = np.asarray(edge_index[0], np.int64)
    col = np.asarray(edge_index[1], np.int64)

    deg = np.bincount(col, minlength=n).astype(np.float32) + 1.0
    dinv = (1.0 / np.sqrt(deg)).astype(np.float32)
    wt = ((1.0 - ALPHA) * dinv[row] * dinv[col]).astype(np.float32)

    # append self loops as explicit edges
    allv = np.arange(n, dtype=np.int64)
    row_a = np.concatenate([row, allv])
    col_a = np.concatenate([col, allv])
    wt_a = np.concatenate([wt, ((1.0 - ALPHA) * dinv * dinv).astype(np.float32)])

    # source window (quarter) + table row within window
    r_s = row_a // n_per
    i_s = row_a % n_per
    q_s = np.minimum(i_s // qs, 3)
    w_s = q_s
    slot = (r_s * qsz[q_s] + (i_s - q_s * qs)).astype(np.int64)

    r_d = col_a // n_per
    dloc = col_a % n_per
    b_idx = dloc // CW                           # dst cell block
    dcol = (dloc % CW).astype(np.int64)

    NB = cfg.NC2
    counts = np.zeros((NCORES, NW, NB), np.int64)
    np.add.at(counts, (r_d, w_s, b_idx), 1)
    n_cb = counts.max(axis=0)                    # [NW, NB]
    n_cb = np.maximum(32 * ((n_cb + 31) // 32), 32)
    L = n_cb.sum(axis=1)
    n_cb[:, NB - 1] += (-L) % P                  # slot streams multiple of 128
    L = n_cb.sum(axis=1)                         # [NW]

    offs = np.zeros((NW, NB + 1), np.int64)
    offs[:, 1:] = np.cumsum(n_cb, axis=1)

    key = (r_d * NW + w_s) * NB + b_idx
    order = np.argsort(key, kind="stable")
    sk = key[order]
    grp_first = np.r_[0, np.flatnonzero(np.diff(sk)) + 1]
    grp_id = np.zeros(len(sk), np.int64)
    grp_id[grp_first[1:]] = 1
    grp_id = np.cumsum(grp_id)
    rank_in_cell = np.arange(len(sk)) - grp_first[grp_id]
    pos = offs[w_s[order], b_idx[order]] + rank_in_cell

    in_maps = []
    for r in range(NCORES):
        m = {}
        xs = np.zeros((cfg.n_pad, cfg.in_dim), np.float32)
        xs[:n_per] = np.asarray(x[r * n_per:(r + 1) * n_per], np.float32)
        m["x"] = xs
        sel_r = r_d[order] == r
        for c in range(NW):
            sel = sel_r & (w_s[order] == c)
            p = pos[sel]
            idx_arr = np.zeros((P, int(L[c]) // P), np.int32)
            idx_arr[p % P, p // P] = slot[order][sel].astype(np.int32)
            sarr = np.zeros((P, (L[c] // P) * CW), H_DT)
            sarr[p % P, (p // P) * CW + dcol[order][sel]] = wt_a[order][sel]
            m[f"idx{c}"] = idx_arr
            m[f"sblk{c}"] = sarr
        m["w_in"] = np.asarray(W_in, np.float32)
        m["b_in"] = np.asarray(b_in, np.float32).reshape(cfg.hid, 1)
        weff = np.concatenate(
            [((1.0 - cfg.betas[i]) * np.eye(cfg.hid, dtype=np.float32)
              + cfg.betas[i] * np.asarray(convs_W[i], np.float32))
             for i in range(cfg.layers)], axis=1)
        m["w_eff"] = weff
        m["w_out"] = np.asarray(W_out, np.float32).reshape(cfg.hid, 1)
        m["b_out"] = np.asarray(b_out, np.float32).reshape(1, 1)
        in_maps.append(m)

    return in_maps, {"n_cb": n_cb, "L": L}


# ---------------- device program ----------------

def build(cfg, meta, debug=False):
    import concourse.bass as bass
    import concourse.mybir as mybir
    from concourse import bacc
    from concourse.masks import make_identity
    from concourse.tile import TileContext
    from contextlib import ExitStack

    f32 = mybir.dt.float32
    f16 = mybir.dt.float16
    i32 = mybir.dt.int32
    Relu = mybir.ActivationFunctionType.Relu
    n_cb, L = meta["n_cb"], meta["L"]
    hid, in_dim = cfg.hid, cfg.in_dim
    qsz = cfg.qsizes

    nc = bacc.Bacc("TRN2", target_bir_lowering=False, debug=debug)

    x_in = nc.dram_tensor("x", [cfg.n_pad, in_dim], f32, kind="ExternalInput")
    idx_in, s_in = [], []
    for c in range(NW):
        idx_in.append(nc.dram_tensor(f"idx{c}", [P, int(L[c]) // P], i32,
                                     kind="ExternalInput"))
        s_in.append(nc.dram_tensor(f"sblk{c}", [P, (int(L[c]) // P) * CW], f16,
                                   kind="ExternalInput"))
    w_in_t = nc.dram_tensor("w_in", [in_dim, hid], f32, kind="ExternalInput")
    b_in_t = nc.dram_tensor("b_in", [hid, 1], f32, kind="ExternalInput")
    w_eff_t = nc.dram_tensor("w_eff", [hid, cfg.layers * hid], f32,
                             kind="ExternalInput")
    w_out_t = nc.dram_tensor("w_out", [hid, 1], f32, kind="ExternalInput")
    b_out_t = nc.dram_tensor("b_out", [1, 1], f32, kind="ExternalInput")
    y_out = nc.dram_tensor("y", [1, cfg.n_pad], f32, kind="ExternalOutput")

    rg = [list(range(NCORES))]

    with TileContext(nc) as tc, ExitStack() as ctx:
        const = ctx.enter_context(tc.tile_pool(name="const", bufs=1))
        gathp = ctx.enter_context(tc.tile_pool(name="gath", bufs=56))
        sblkp = ctx.enter_context(tc.tile_pool(name="sblk", bufs=3))
        idxp = ctx.enter_context(tc.tile_pool(name="idxt", bufs=2))
        xiop = ctx.enter_context(tc.tile_pool(name="xio", bufs=2))
        xtp = ctx.enter_context(tc.tile_pool(name="xt", bufs=3))
        wbp = ctx.enter_context(tc.tile_pool(name="wb", bufs=3))
        hbp = ctx.enter_context(tc.tile_pool(name="hb", bufs=2))
        h4p = ctx.enter_context(tc.tile_pool(name="h4", bufs=2))
        ytp = ctx.enter_context(tc.tile_pool(name="yt", bufs=2))
        pagg = ctx.enter_context(tc.tile_pool(name="pagg", bufs=3, space="PSUM"))
        ptr = ctx.enter_context(tc.tile_pool(name="ptr", bufs=3, space="PSUM"))
        pmisc = ctx.enter_context(tc.tile_pool(name="pmisc", bufs=2, space="PSUM"))
        dram = ctx.enter_context(tc.tile_pool(name="dram", bufs=1, space="DRAM"))

        # node-major per-quarter h shard + AG'd tables (fp16)
        h_shard = []
        h_table = [None] * 4
        for q in range(4):
            h_shard.append(dram.tile([qsz[q], P], f16, tag=f"h_shard{q}",
                                     name=f"h_shard{q}"))

        id_f = const.tile([P, P], f32, tag="id_f", name="id_f")
        make_identity(nc, id_f[:])
        w_in_sb = const.tile([P, (in_dim // P) * hid], f32, tag="w_in",
                             name="w_in_sb")
        for k in range(in_dim // P):
            nc.sync.dma_start(out=w_in_sb[:, k * hid:(k + 1) * hid],
                              in_=w_in_t[k * P:(k + 1) * P, :])
        b_in_sb = const.tile([P, 1], f32, tag="b_in", name="b_in_sb")
        nc.sync.dma_start(out=b_in_sb[:], in_=b_in_t[:])
        b_in_s = const.tile([P, 1], f32, tag="b_in_s", name="b_in_s")
        nc.vector.tensor_scalar_mul(b_in_s[:], b_in_sb[:], ALPHA)
        w_eff_sb = const.tile([P, cfg.layers * hid], f32, tag="w_eff",
                              name="w_eff_sb")
        nc.sync.dma_start(out=w_eff_sb[:], in_=w_eff_t[:])
        w_out_sb = const.tile([P, 1], f32, tag="w_out", name="w_out_sb")
        nc.sync.dma_start(out=w_out_sb[:], in_=w_out_t[:])
        b_out_sb = const.tile([1, 1], f32, tag="b_out", name="b_out_sb")
        nc.sync.dma_start(out=b_out_sb[:], in_=b_out_t[:])

        # SBUF-resident accumulators
        s_sb = const.tile([P, cfg.n_pad], f32, tag="s_sb", name="s_sb")
        h0a_sb = const.tile([P, cfg.n_pad], f32, tag="h0a", name="h0a_sb")

        def emit_ag(q):
            tab = dram.tile([NCORES * qsz[q], P], f16, tag=f"h_table{q}",
                            name=f"h_table{q}", addr_space="Shared", bufs=2)
            nc.gpsimd.collective_compute(
                "AllGather", mybir.AluOpType.bypass, replica_groups=rg,
                ins=[h_shard[q][:, :].opt()],
                outs=[tab[:].opt()])
            h_table[q] = tab

        def write_h_rows(hs4, nblk, lo):
            """DMA hs4[:, :nblk, :] (node%128, blk, feat) to node-major
            h_shard rows [lo, lo+128*nblk), splitting at quarter bounds."""
            done = 0
            while done < nblk:
                pos = lo + done * P
                q = min(pos // cfg.qs, 3)
                co = pos - q * cfg.qs
                take = min(nblk - done, (qsz[q] - co) // P)
                if take <= 0:
                    break                 # pad rows past the real nodes
                dst = h_shard[q][co:co + take * P, :].rearrange(
                    "(j p) d -> p j d", p=P)
                nc.sync.dma_start(out=dst, in_=hs4[:, done:done + take, :])
                done += take

        def fire_ags(lo, hi):
            for q in range(3):
                if lo < (q + 1) * cfg.qs <= hi:
                    emit_ag(q)
            if hi >= cfg.n_pad:
                emit_ag(3)

        # ---------------- init: h0 = relu(x@W_in + b_in) ----------------
        nblk_grp = 4
        for nt0 in range(0, cfg.NB, nblk_grp):
            nb = min(nblk_grp, cfg.NB - nt0)
            hs4 = h4p.tile([P, nblk_grp, P], f16, tag="hs4", name="hs4")
            x4 = xiop.tile([P, nblk_grp, in_dim], f32, tag="x", name="x4")
            nc.sync.dma_start(
                out=x4[:, :nb, :],
                in_=x_in[nt0 * P:(nt0 + nb) * P, :].rearrange(
                    "(j p) d -> p j d", p=P))
            for j in range(nb):
                nt = nt0 + j
                x_tile = x4[:, j, :]
                xts = []
                for k in range(in_dim // P):
                    xt_ps = ptr.tile([P, P], f32, tag="ptr", name="xt_ps")
                    nc.tensor.transpose(xt_ps[:], x_tile[:, k * P:(k + 1) * P],
                                        id_f[:])
                    xt_sb = xtp.tile([P, P], f32, tag="xt", name="xt_sb")
                    nc.vector.tensor_copy(out=xt_sb[:], in_=xt_ps[:])
                    xts.append(xt_sb)
                ph0 = pmisc.tile([P, YB], f32, tag="pmisc", name="pm")
                nk = in_dim // P
                for k in range(nk):
                    nc.tensor.matmul(ph0[:, :P],
                                     lhsT=w_in_sb[:, k * hid:(k + 1) * hid],
                                     rhs=xts[k][:], start=(k == 0),
                                     stop=(k == nk - 1))
                # alpha * h0 stays feat-major in SBUF
                nc.scalar.activation(h0a_sb[:, nt * P:(nt + 1) * P], ph0[:, :P],
                                     Relu, bias=b_in_s[:], scale=ALPHA)
                # h -> transpose to node-major, cast fp16 on copy-out
                hb = wbp.tile([P, P], f32, tag="wb", name="hbi")
                nc.scalar.activation(hb[:], ph0[:, :P], Relu, bias=b_in_sb[:])
                pt = ptr.tile([P, P], f32, tag="ptr", name="pt")
                nc.tensor.transpose(pt[:], hb[:], id_f[:])
                nc.vector.tensor_copy(out=hs4[:, j, :], in_=pt[:])
            write_h_rows(hs4, nb, nt0 * P)
            fire_ags(nt0 * P, (nt0 + nb) * P)

        # ---------------- layers ----------------
        for layer in range(cfg.layers):
            last = layer == cfg.layers - 1
            tables = list(h_table)
            pending_ags = []      # (quarter, window3-cell to fire after)

            def finish_block(b, tables=tables, layer=layer, last=last,
                             pending_ags=pending_ags):
                """YB-wide output block b of s_sb is complete"""
                w = min(YB, cfg.n_pad - b * YB)
                cols = slice(b * YB, b * YB + w)
                ps = pmisc.tile([P, YB], f32, tag="pmisc", name="pm")
                nc.tensor.matmul(ps[:, :w],
                                 lhsT=w_eff_sb[:, layer * hid:(layer + 1) * hid],
                                 rhs=s_sb[:, cols], start=True, stop=True)
                if not last:
                    hb = hbp.tile([P, YB], f32, tag="hb", name="hb")
                    nc.scalar.activation(hb[:, :w], ps[:, :w], Relu)
                    hs4 = h4p.tile([P, YB // P, P], f16, tag="hs4f", name="hs4f")
                    for j in range(w // P):
                        pt = ptr.tile([P, P], f32, tag="ptr", name="pt")
                        nc.tensor.transpose(pt[:], hb[:, j * P:(j + 1) * P],
                                            id_f[:])
                        nc.vector.tensor_copy(out=hs4[:, j, :], in_=pt[:])
                    write_h_rows(hs4, w // P, b * YB)
                    # defer AG issuance a few cells so its SEQ wait (on the
                    # h_shard write chain) resolves before it can stall the
                    # Pool queue's gather stream
                    lo, hi = b * YB, b * YB + w
                    cur_cell = (hi - 1) // CW
                    for q in range(3):
                        if lo < (q + 1) * cfg.qs <= hi:
                            pending_ags.append((q, cur_cell + 6))
                    if hi >= cfg.n_pad:
                        pending_ags.append((3, cur_cell + 6))
                else:
                    h8 = wbp.tile([P, YB], f32, tag="wb", name="wb")
                    nc.scalar.activation(h8[:, :w], ps[:, :w], Relu)
                    psy = pmisc.tile([P, YB], f32, tag="pmisc", name="pm")
                    nc.tensor.matmul(psy[0:1, :w], lhsT=w_out_sb[:, 0:1],
                                     rhs=h8[:, :w], start=True, stop=True)
                    yt = ytp.tile([1, YB], f32, tag="yt", name="yt")
                    nc.vector.tensor_tensor(
                        out=yt[0:1, :w], in0=psy[0:1, :w],
                        in1=b_out_sb[0:1, 0:1].to_broadcast([1, w]),
                        op=mybir.AluOpType.add)
                    nc.sync.dma_start(out=y_out[0:1, b * YB:b * YB + w],
                                      in_=yt[0:1, :w])

            # prefetch every window's gather-index tile up front
            it_wins = []
            for c in range(NW):
                it_win = idxp.tile([P, int(L[c]) // P], i32, tag=f"itw{c}",
                                   name="it_win")
                nc.sync.dma_start(out=it_win[:], in_=idx_in[c][:])
                it_wins.append(it_win)

            for c in range(NW):
                Lc = int(L[c])
                ng = Lc // P
                it_win = it_wins[c]
                gt_tiles = {}
                s_tiles = [None] * ((ng + SB - 1) // SB)

                def ensure_group(g, c=c, it_win=it_win, gt_tiles=gt_tiles,
                                 tables=tables):
                    if g in gt_tiles:
                        return
                    gt = gathp.tile([P, P], f16, tag="gt", name="gt")
                    nc.gpsimd.indirect_dma_start(
                        out=gt[:], out_offset=None,
                        in_=tables[c][:],
                        in_offset=bass.IndirectOffsetOnAxis(
                            ap=it_win[:, g:g + 1], axis=0))
                    gt_tiles[g] = gt

                def ensure_s(sb, c=c, ng=ng, s_tiles=s_tiles):
                    if s_tiles[sb] is not None:
                        return
                    st = sblkp.tile([P, SB * CW], f16, tag="st", name="st")
                    lo = sb * SB * CW
                    ncols = min(SB * CW, ng * CW - lo)
                    nc.sync.dma_start(out=st[:, :ncols],
                                      in_=s_in[c][:, lo:lo + ncols])
                    s_tiles[sb] = st

                cur = 0
                for b in range(cfg.NC2):
                    ps_b = pagg.tile([P, CW], f32, tag="ps_b", name="ps_b")
                    n_slots = int(n_cb[c][b])
                    first = True
                    left = n_slots
                    while left > 0:
                        g, p0 = cur // P, cur % P
                        ln = 0
                        for sz in (128, 64, 32):
                            if p0 % sz == 0 and left >= sz and p0 + sz <= P:
                                ln = sz
                                break
                        assert ln, (p0, left)
                        ensure_group(g)
                        ensure_s(g // SB)
                        st = s_tiles[g // SB]
                        so = (g % SB) * CW
                        nc.tensor.matmul(
                            ps_b[:],
                            lhsT=gt_tiles[g][p0:p0 + ln, :],
                            rhs=st[p0:p0 + ln, so:so + CW],
                            start=first, stop=(ln == left),
                            tile_position=(p0, 0))
                        first = False
                        cur += ln
                        left -= ln
                    cols = slice(b * CW, (b + 1) * CW)
                    if c == 0:
                        nc.vector.tensor_add(out=s_sb[:, cols],
                                             in0=h0a_sb[:, cols], in1=ps_b[:])
                    else:
                        nc.vector.tensor_add(out=s_sb[:, cols],
                                             in0=s_sb[:, cols], in1=ps_b[:])
                    if c == NW - 1:
                        while pending_ags and pending_ags[0][1] <= b:
                            emit_ag(pending_ags.pop(0)[0])
                        if (((b + 1) * CW) % YB == 0 or b == cfg.NC2 - 1):
                            finish_block((b * CW) // YB)
            while pending_ags:
                emit_ag(pending_ags.pop(0)[0])

    nc.compile()
    return nc


# ---------------- top level ----------------

def _assemble_y(results, cfg):
    parts = []
    for r in range(NCORES):
        y = np.asarray(results[r]["y"], np.float32).reshape(-1)
        parts.append(y[:cfg.n_per])
    return np.concatenate(parts)


def _run_pjrt(nc, in_maps, n_cores, time_iters=0, devices=None, donate=True):
    """Execute the bass program on the NeuronCores via PJRT (the axon
    redirect path of run_bass_kernel_spmd), with inputs pre-staged on
    device.  Mirrors concourse.bass2jax.run_bass_via_pjrt (multi-core).

    The axon dispatch floor is ~80ms/call, so single-call wall time says
    nothing about device time; with time_iters > 0 the marginal cost per
    execute between pipelined batches of M_lo and M_hi back-to-back
    calls is reported: device exec time plus ~1ms per-call dispatch (an
    honest upper bound on HW time).
    """
    import time
    import jax
    from jax.sharding import Mesh, NamedSharding, PartitionSpec
    from jax.experimental.shard_map import shard_map
    from concourse import bass2jax, mybir

    bass2jax.install_neuronx_cc_hook()

    partition_name = nc.partition_id_tensor.name if nc.partition_id_tensor else None
    in_names, out_names, out_avals, zero_outs = [], [], [], []
    for alloc in nc.m.functions[0].allocations:
        if not isinstance(alloc, mybir.MemoryLocationSet):
            continue
        name = alloc.memorylocations[0].name
        if alloc.kind == "ExternalInput":
            if name != partition_name:
                in_names.append(name)
        elif alloc.kind == "ExternalOutput":
            out_names.append(name)
            shape = tuple(alloc.tensor_shape)
            dtype = mybir.dt.np(alloc.dtype)
            out_avals.append(jax.core.ShapedArray(shape, dtype))
            zero_outs.append(np.zeros(shape, dtype))
    n_params = len(in_names)
    n_outs = len(out_avals)
    in_names.extend(out_names)
    if partition_name is not None:
        in_names.append(partition_name)
    donate = tuple(range(n_params, n_params + n_outs)) if donate else ()

    def _body(*args):
        operands = list(args)
        if partition_name is not None:
            operands.append(bass2jax.partition_id_tensor())
        outs = bass2jax._bass_exec_p.bind(
            *operands,
            out_avals=tuple(out_avals),
            in_names=tuple(in_names),
            out_names=tuple(out_names),
            lowering_input_output_aliases=(),
            sim_require_finite=True,
            sim_require_nnan=True,
            nc=nc,
        )
        return tuple(outs)

    if devices is None:
        devices = jax.devices()[:n_cores]
    assert len(devices) == n_cores
    mesh = Mesh(np.asarray(devices), ("core",))
    in_specs = (PartitionSpec("core"),) * (n_params + n_outs)
    out_specs = (PartitionSpec("core"),) * len(out_names)
    sharded = jax.jit(
        shard_map(_body, mesh=mesh, in_specs=in_specs, out_specs=out_specs,
                  check_rep=False),
        donate_argnums=donate, keep_unused=True)

    shard = NamedSharding(mesh, PartitionSpec("core"))
    concat_in = [
        jax.device_put(
            np.concatenate([np.asarray(in_maps[c][name]) for c in range(n_cores)],
                           axis=0), shard)
        for name in in_names[:n_params]
    ]
    jax.block_until_ready(concat_in)

    def zeros():
        return [
            jax.device_put(np.zeros((n_cores * z.shape[0], *z.shape[1:]), z.dtype),
                           shard)
            for z in zero_outs
        ]

    out_arrs = jax.block_until_ready(sharded(*concat_in, *zeros()))
    exec_ns = None
    if time_iters > 0:
        m_lo, m_hi = 4, 4 + max(4, time_iters)

        def run_m(m):
            zs = [zeros() for _ in range(m)]
            jax.block_until_ready(zs)
            t0 = time.perf_counter()
            rs = [sharded(*concat_in, *z) for z in zs]
            jax.block_until_ready(rs)
            return time.perf_counter() - t0

        run_m(2)  # warm
        lo = min(run_m(m_lo) for _ in range(2))
        hi = min(run_m(m_hi) for _ in range(2))
        exec_ns = int(max(hi - lo, 0) / (m_hi - m_lo) * 1e9)
    results = [
        {name: np.asarray(out_arrs[i]).reshape(n_cores, *out_avals[i].shape)[c]
         for i, name in enumerate(out_names)}
        for c in range(n_cores)
    ]
    return results, exec_ns


def _kernel_impl(inputs, cfg, devices=None, donate=True, iters=0):
    in_maps, meta = preprocess(cfg=cfg, **inputs)
    nc = build(cfg, meta)
    results, exec_ns = _run_pjrt(nc, in_maps, NCORES, time_iters=iters,
                                 devices=devices, donate=donate)
    return _assemble_y(results, cfg), exec_ns


def kernel(x, edge_index, W_in, b_in, convs_W, W_out, b_out):
    global LAST_EXEC_NS
    iters = int(os.environ.get("KERNEL_TIME_ITERS", "0"))
    y, exec_ns = _kernel_impl(
        dict(x=x, edge_index=edge_index, W_in=W_in, b_in=b_in,
             convs_W=convs_W, W_out=W_out, b_out=b_out),
        _cfg_full(), iters=iters)
    LAST_EXEC_NS = exec_ns
    return y
